# revision 31
# baseline (speedup 1.0000x reference)
"""GSA video block kernel for 8 TRN2 NeuronCores (v2).

Sharding: head-parallel attention (2 heads/core) -> one AllToAll that
redistributes the RMS-normed head outputs from head-sharded to
token-sharded -> token-parallel tail (out-proj + LN2 + MLP with full
weights, 256 tokens/core).

v2 layout strategy: the host supplies x TRANSPOSED (xT [D, TOK]), so
LN1 never materializes h -- projections run feat-major from xT with a
rank-1 mean correction accumulated into the psum and a broadcast-rstd
multiply afterwards.  Token-major k/v/f come from tensor-engine
transposes (no DMA transposes anywhere).  The chunked scan processes
the 4 batches' chunk-c tiles as single [128,512]-wide ops, with all
state-independent prep emitted ahead of the serial state chain.  The
MLP computes y1 m-major so MLP2 needs no transpose.
"""

import os
import sys

import numpy as np
import ml_dtypes

if "/opt/trn_rl_repo" not in sys.path:
    sys.path.insert(0, "/opt/trn_rl_repo")

import concourse.bass as bass  # noqa: E402
import concourse.mybir as mybir  # noqa: E402
import concourse.tile as tile  # noqa: E402
from concourse import bacc  # noqa: E402
from concourse.bass_utils import run_bass_kernel_spmd  # noqa: E402

BF16 = mybir.dt.bfloat16
F32 = mybir.dt.float32
AF = mybir.ActivationFunctionType
ALU = mybir.AluOpType
AX = mybir.AxisListType

B, T, D = 4, 512, 1024
H, DK, DV, M = 16, 64, 64, 64
MLP = 4096
EPS = 1e-6

N_CORES = 8
C = 128                    # scan chunk length
NCH = T // C               # chunks per batch = 4
TOK = B * T                # 2048 flat tokens
TT = TOK // 128            # 16 token tiles
DT = D // 128              # 8 d tiles
MT = MLP // 128            # 32 mlp tiles
TAIL = TOK // N_CORES      # 256 tokens per core in the tail
RG = [list(range(N_CORES))]
P = 128

_cache = {}


def _emit(nc, tc, io):
    xT, x_res = io["xT"], io["x_res"]
    wqkvf, wo, w1, w2 = io["wqkvf"], io["wo"], io["w1"], io["w2"]
    blob_f32, blob_bf16, rowblob = io["blob_f32"], io["blob_bf16"], io["rowblob"]
    y_out, dump = io["y_out"], io["dump"]

    const = tc.alloc_tile_pool(name="const", bufs=1)
    dram = tc.alloc_tile_pool(name="dram", bufs=1, space="DRAM")

    # ---- warmup collective (prepay ncfw handshake) -----------------------
    wa_in = dram.tile([8, 128], BF16, name="wa_in")
    wa_out = dram.tile([8, 128], BF16, name="wa_out")
    nc.gpsimd.collective_compute("AllReduce", ALU.add, replica_groups=RG,
                                 ins=[wa_in.opt()], outs=[wa_out.opt()])
    wa2_in = dram.tile([8, 1024], BF16, name="wa2_in")
    wa2_out = dram.tile([8, 1024], BF16, name="wa2_out")
    nc.gpsimd.collective_compute("AllToAll", ALU.bypass, replica_groups=RG,
                                 ins=[wa2_in.opt()], outs=[wa2_out.opt()])

    a2a_in = dram.tile([128 * N_CORES, TAIL], BF16, name="a2a_in")
    a2a_out = dram.tile([128 * N_CORES, TAIL], BF16, name="a2a_out")

    # ---- constants into SBUF (SP queue) ---------------------------------
    cf = const.tile([128, 129], F32, name="cf")
    nc.sync.dma_start(cf[:], blob_f32.ap())
    ltriT_sb = cf[:, 0:128]
    onescol = cf[:, 128:129]                              # all ones
    cb = const.tile([128, 3, 128], BF16, name="cb")       # cmask|ident|bd128
    nc.sync.dma_start(cb[:], blob_bf16.ap().rearrange("p (n f) -> p n f", n=3))
    cmask_bc = cb[:, 0:1, :].to_broadcast([128, 4, 128])
    ident_sb, bd128_sb = cb[:, 1, :], cb[:, 2, :]
    rows = const.tile([1, 10, 128], BF16, name="rows")
    nc.sync.dma_start(rows[:], rowblob.ap().rearrange("o (n f) -> o n f", n=10))
    csq_r, csk_r, csv_r, csf_r = (rows[:, i, :] for i in range(4))
    ones_row = rows[:, 8, :]
    one1 = rows[:, 9, 0:1]
    bcol = const.tile([128, 4], F32, name="bcol")
    nc.sync.dma_start(bcol[:], io["bcols"].ap())
    eps_sb = const.tile([128, 1], F32)
    nc.vector.memset(eps_sb[:], EPS)
    eps2_sb = const.tile([128, 1], F32)
    nc.vector.memset(eps2_sb[:], 1e-30)

    # ---- persistent activation tensors (allocated below xp/wp in the
    # pool stack; released only at the very end) --------------------------
    persist = tc.alloc_tile_pool(name="persist", bufs=1)
    qT = persist.tile([128, TOK], BF16, name="qT")
    kT = persist.tile([128, TOK], BF16, name="kT")
    k_tm = persist.tile([128, TT, 128], BF16, name="k_tm")
    v_tm = persist.tile([128, TT, 128], BF16, name="v_tm")
    sp = persist.tile([128, TT, 128], F32, name="sp")
    s_tm = persist.tile([128, TT, 128], BF16, name="s_tm")
    onT = persist.tile([128, TOK], BF16, name="onT")

    # ---- bulk loads -----------------------------------------------------
    xp = tc.alloc_tile_pool(name="xp", bufs=1)
    xT_sb = xp.tile([128, DT, TOK], BF16, name="xT_sb")
    for dt in range(DT):
        nc.sync.dma_start(xT_sb[:, dt, :],
                          xT.ap().rearrange("(dt p) t -> p dt t", p=P)[:, dt, :])
    wp = tc.alloc_tile_pool(name="wp", bufs=1)
    w4_sb = wp.tile([128, DT, 512], BF16, name="w4_sb")   # wq|wk|wv|wf cols
    nc.scalar.dma_start(w4_sb[:], wqkvf.ap().rearrange("(dt p) j -> p dt j", p=P))
    wo_sb = const.tile([128, DT, D], BF16, name="wo_sb")
    nc.scalar.dma_start(wo_sb[:], wo.ap().rearrange("(dt p) j -> p dt j", p=P))
    x2_sb = const.tile([128, 2, D], F32, name="x2_sb")
    nc.sync.dma_start(x2_sb[:], x_res.ap().rearrange("(n p) d -> p n d", p=P))

    # =====================================================================
    # P1: LN1 stats from xT via matmul reduction (per 512-token group)
    # =====================================================================
    stats = tc.alloc_tile_pool(name="stats", bufs=1)
    mu_r = stats.tile([1, 4, 512], BF16, name="mu_r")
    sd_r = stats.tile([1, 4, 512], BF16, name="sd_r")
    rbc = stats.tile([128, 4, 512], F32, name="rbc")
    oc = stats.tile([128, 1], BF16, name="oc")
    nc.vector.memset(oc[:], 1.0 / D)
    with tc.tile_pool(name="sq_p", bufs=3) as sqp, \
         tc.tile_pool(name="st_ps", bufs=2, space="PSUM") as stps, \
         tc.tile_pool(name="rb_ps", bufs=2, space="PSUM") as rbps, \
         tc.tile_pool(name="st_sb", bufs=2) as stsb:
        for g in range(4):
            gsl = slice(g * 512, (g + 1) * 512)
            ps_mu = stps.tile([1, 512], F32, name="ps_mu")
            ps_e2 = stps.tile([1, 512], F32, name="ps_e2")
            for dt in range(DT):
                sq = sqp.tile([128, 512], BF16, name="sq")
                nc.vector.tensor_tensor(sq[:], xT_sb[:, dt, gsl],
                                        xT_sb[:, dt, gsl], ALU.mult)
                nc.tensor.matmul(ps_mu[:], oc[:], xT_sb[:, dt, gsl],
                                 start=(dt == 0), stop=(dt == DT - 1))
                nc.tensor.matmul(ps_e2[:], oc[:], sq[:],
                                 start=(dt == 0), stop=(dt == DT - 1))
            # var = e2 - mu^2 ; sd = sqrt(var+eps); rbc = 1/bcast(sd)
            nc.scalar.activation(mu_r[:, g, :], ps_mu[:], AF.Copy)
            var = stsb.tile([1, 512], F32, name="var")
            nc.vector.tensor_tensor(var[:], mu_r[:, g, :], mu_r[:, g, :],
                                    ALU.mult)
            nc.vector.tensor_tensor(var[:], ps_e2[:], var[:], ALU.subtract)
            nc.scalar.activation(sd_r[:, g, :], var[:], AF.Sqrt,
                                 bias=eps_sb[0:1, :])
            ps_rb = rbps.tile([128, 512], F32, name="ps_rb")
            nc.tensor.matmul(ps_rb[:], ones_row, sd_r[:, g, :],
                             start=True, stop=True)
            nc.vector.reciprocal(rbc[:, g, :], ps_rb[:])

    # =====================================================================
    # P2: projections, feat-major, with rank-1 mean correction
    # =====================================================================
    ffeat = tc.alloc_tile_pool(name="ffeat", bufs=1)
    f_ft = ffeat.tile([128, TOK], BF16, name="f_ft")
    with tc.tile_pool(name="pj_ps", bufs=3, space="PSUM") as pjps, \
         tc.tile_pool(name="pj_sb", bufs=3) as pjsb, \
         tc.tile_pool(name="tr_ps", bufs=2, space="PSUM") as trps:
        def proj_psum(jsl, cs_row, g):
            gsl = slice(g * 512, (g + 1) * 512)
            bank = pjps.tile([128, 512], F32, name="pjbank")
            for dt in range(DT):
                nc.tensor.matmul(bank[:], w4_sb[:, dt, jsl], xT_sb[:, dt, gsl],
                                 start=(dt == 0), stop=False)
            nc.tensor.matmul(bank[:], cs_row, mu_r[:, g, :],
                             start=False, stop=True)
            return bank

        for g in range(4):                      # q: silu -> qT
            bank = proj_psum(slice(0, 128), csq_r, g)
            gsl = slice(g * 512, (g + 1) * 512)
            yp = pjsb.tile([128, 512], BF16, name="yp")
            nc.vector.tensor_tensor(yp[:], bank[:], rbc[:, g, :], ALU.mult)
            nc.scalar.activation(qT[:, gsl], yp[:], AF.Silu,
                                 bias=bcol[:, 0:1])
        for g in range(4):                      # k: silu -> kT (+ k_tm below)
            bank = proj_psum(slice(128, 256), csk_r, g)
            gsl = slice(g * 512, (g + 1) * 512)
            yp = pjsb.tile([128, 512], BF16, name="yp")
            nc.vector.tensor_tensor(yp[:], bank[:], rbc[:, g, :], ALU.mult)
            nc.scalar.activation(kT[:, gsl], yp[:], AF.Silu,
                                 bias=bcol[:, 1:2])
        for g in range(4):
            pst = trps.tile([128, 4, 128], BF16, name="pst")
            for cc in range(4):
                ti = g * 4 + cc
                nc.tensor.transpose(pst[:, cc, :],
                                    kT[:, ti * 128:(ti + 1) * 128], ident_sb)
            nc.vector.tensor_copy(k_tm[:, g * 4:(g + 1) * 4, :], pst[:])
        for g in range(4):                      # v: mult only -> v_tm
            bank = proj_psum(slice(256, 384), csv_r, g)
            yp = pjsb.tile([128, 512], BF16, name="yp")
            nc.vector.tensor_tensor(yp[:], bank[:], rbc[:, g, :], ALU.mult)
            pst = trps.tile([128, 4, 128], BF16, name="pst")
            for cc in range(4):
                nc.tensor.transpose(pst[:, cc, :], yp[:, cc * 128:(cc + 1) * 128],
                                    ident_sb)
            nc.vector.tensor_copy(v_tm[:, g * 4:(g + 1) * 4, :], pst[:])
        for g in range(4):                      # f -> f_ft (feat-major)
            bank = proj_psum(slice(384, 512), csf_r, g)
            gsl = slice(g * 512, (g + 1) * 512)
            nc.vector.tensor_tensor(f_ft[:, gsl], bank[:], rbc[:, g, :],
                                    ALU.mult)
        # gates: sp = softplus(-f) = ln(1+exp(-f)); three passes so the
        # activation table switches exp -> ln -> exp only once each
        enf_a = pjsb.tile([128, TT, 128], F32, name="enf_a", bufs=1)
        for g in range(4):
            pst = trps.tile([128, 4, 128], BF16, name="pst")
            for cc in range(4):
                ti = g * 4 + cc
                nc.tensor.transpose(pst[:, cc, :],
                                    f_ft[:, ti * 128:(ti + 1) * 128], ident_sb)
            csl = slice(g * 4, (g + 1) * 4)
            nc.scalar.activation(enf_a[:, csl, :], pst[:], AF.Exp, scale=-1.0)
        for g in range(4):
            csl = slice(g * 4, (g + 1) * 4)
            nc.scalar.activation(sp[:, csl, :], enf_a[:, csl, :], AF.Ln,
                                 bias=1.0)
        for g in range(4):
            csl = slice(g * 4, (g + 1) * 4)
            e8 = pjsb.tile([128, 4, 128], BF16, name="e8")
            nc.scalar.activation(e8[:], sp[:, csl, :], AF.Exp, scale=-0.125)
            nc.vector.tensor_scalar(s_tm[:, csl, :], e8[:], -1.0, 1.0,
                                    ALU.mult, ALU.add)
    ffeat.release()
    stats.release()
    wp.release()
    xp.release()

    # start w1 prefetch now (ACT queue), in mlp-dim chunks so MLP1 can
    # start consuming the first mt tiles as soon as they land
    w1p = tc.alloc_tile_pool(name="w1p", bufs=1)
    w1_sb = w1p.tile([128, DT, MLP], BF16, name="w1_sb")
    for q4 in range(4):
        msl = slice(q4 * 1024, (q4 + 1) * 1024)
        nc.scalar.dma_start(
            w1_sb[:, :, msl],
            w1.ap().rearrange("(dt p) m -> p dt m", p=P)[:, :, msl])

    for nm, t_sb in (("qT", qT), ("kT", kT)):
        if (d := dump(nm, [128, TOK], BF16)) is not None:
            nc.sync.dma_start(d.ap(), t_sb[:])
    for nm, t_sb in (("k_tm", k_tm), ("v_tm", v_tm), ("s_tm", s_tm)):
        if (d := dump(nm, [128, TT * 128], BF16)) is not None:
            nc.sync.dma_start(d.ap().rearrange("p (n f) -> p n f", n=TT), t_sb[:])
    if (d := dump("sp", [128, TT * 128])) is not None:
        nc.sync.dma_start(d.ap().rearrange("p (n f) -> p n f", n=TT), sp[:])

    # =====================================================================
    # P3: chunked scan.  Token tile index = b*4+c.  Group = fixed c, 4 b's.
    # =====================================================================
    scank = tc.alloc_tile_pool(name="scank", bufs=1)
    Kst = scank.tile([128, 4, 64], BF16, name="Kst")     # [(2h dk), b, m]
    Vst = scank.tile([128, 4, 64], BF16, name="Vst")     # [(2h m), b, dv]
    lam_a = scank.tile([128, NCH, 4, 128], BF16, name="lam_a")
    stil_a = scank.tile([128, NCH, 4, 128], BF16, name="stil_a")
    stT_a = scank.tile([128, NCH, 4, 128], BF16, name="stT_a")
    am_a = scank.tile([128, NCH, 2, 4, 128], BF16, name="am_a")
    dk_a = scank.tile([128, NCH, 4, 64], BF16, name="dk_a")   # [(2h dk), b, m]
    dv_a = scank.tile([128, NCH, 4, 64], BF16, name="dv_a")   # [(2h m), b, dv]
    lbc_a = scank.tile([128, NCH, 4, 128], BF16, name="lbc_a")
    dec_a = scank.tile([128, NCH, 4], BF16, name="dec_a")

    def cgv(t_sb, c):
        # [128, TT, f] -> [128, 4b, f] strided view for chunk c
        return t_sb[:].rearrange("p (b c) f -> p c b f", c=NCH)[:, c]

    # serial-phase psum pool FIRST so its banks are disjoint from prep's
    with tc.tile_pool(name="se_ps", bufs=1, space="PSUM") as seps, \
         tc.tile_pool(name="pr_ps", bufs=3, space="PSUM") as prps, \
         tc.tile_pool(name="pr_sb", bufs=2) as prsb, \
         tc.tile_pool(name="se_sb", bufs=2) as sesb:
        # ---------------- prep (state-independent), all c ----------------
        for c in range(NCH):
            ps_cs = prps.tile([128, 4, 128], F32, name="ps_cs", tag="pband")
            nc.tensor.matmul(ps_cs[:], ltriT_sb[:], cgv(sp, c),
                             start=True, stop=True)
            nc.scalar.activation(lam_a[:, c], ps_cs[:], AF.Exp)
            en4 = prsb.tile([128, 4, 128], BF16, name="en4")
            nc.scalar.activation(en4[:], ps_cs[:], AF.Exp, scale=-1.0)
            ps_ct = prps.tile([1, 4, 128], F32, name="ps_ct", tag="pband")
            nc.tensor.matmul(ps_ct[:], onescol, cgv(sp, c),
                             start=True, stop=True)
            lamCr = prsb.tile([1, 4, 128], BF16, name="lamCr")
            nc.scalar.activation(lamCr[:], ps_ct[:], AF.Exp, scale=-0.125)
            ps_lb = prps.tile([128, 4, 128], F32, name="ps_lb", tag="pband")
            nc.tensor.matmul(ps_lb[:], ones_row,
                             lamCr[:].rearrange("o b f -> o (b f)"),
                             start=True, stop=True)
            nc.scalar.activation(lbc_a[:, c], ps_lb[:], AF.Copy)
            nc.vector.tensor_tensor(stil_a[:, c], cgv(s_tm, c), en4[:],
                                    ALU.mult)
            s2 = prsb.tile([128, 4, 128], BF16, name="s2")
            nc.vector.tensor_tensor(s2[:], stil_a[:, c], ps_lb[:], ALU.mult)
            ps_dc = prps.tile([128, 4], F32, name="ps_dc", tag="pband")
            for b in range(4):
                nc.tensor.matmul(ps_dc[:, b:b + 1], lamCr[:, b, :], one1,
                                 start=True, stop=True)
            nc.scalar.activation(dec_a[:, c], ps_dc[:], AF.Copy)
            for h in range(2):
                hs = slice(h * 64, (h + 1) * 64)
                ps_a = prps.tile([128, 4, 128], F32, name="ps_a", tag="pband")
                for b in range(4):
                    tsl = slice((b * 4 + c) * 128, (b * 4 + c + 1) * 128)
                    nc.tensor.matmul(ps_a[:, b, :], kT[hs, tsl], qT[hs, tsl],
                                     start=True, stop=True)
                nc.vector.tensor_tensor(am_a[:, c, h], ps_a[:], cmask_bc,
                                        ALU.mult)
            ps_st = prps.tile([128, 4, 128], BF16, name="ps_st", tag="pband")
            for b in range(4):
                nc.tensor.transpose(ps_st[:, b, :], stil_a[:, c, b, :],
                                    ident_sb)
            nc.scalar.activation(stT_a[:, c], ps_st[:], AF.Copy)
            ps_dk = prps.tile([128, 4, 64], F32, name="ps_dk", tag="pband")
            for h in range(2):
                hs = slice(h * 64, (h + 1) * 64)
                for b in range(4):
                    bi = b * 4 + c
                    nc.tensor.matmul(ps_dk[hs, b, :], k_tm[:, bi, hs],
                                     s2[:, b, hs], start=True, stop=True)
            nc.scalar.activation(dk_a[:, c], ps_dk[:], AF.Copy)
            ps_dv = prps.tile([128, 4, 64], F32, name="ps_dv", tag="pband")
            for h in range(2):
                hs = slice(h * 64, (h + 1) * 64)
                for b in range(4):
                    bi = b * 4 + c
                    nc.tensor.matmul(ps_dv[hs, b, :], s2[:, b, hs],
                                     v_tm[:, bi, hs], start=True, stop=True)
            nc.scalar.activation(dv_a[:, c], ps_dv[:], AF.Copy)

        # ---------------- serial state chain -----------------------------
        for c in range(NCH):
            first = (c == 0)
            ps_ok = seps.tile([128, 4, 2, 64], F32, name="ps_ok")
            for h in range(2):
                hs = slice(h * 64, (h + 1) * 64)
                for b in range(4):
                    tsl = slice((b * 4 + c) * 128, (b * 4 + c + 1) * 128)
                    if not first:
                        nc.tensor.matmul(ps_ok[:, b, h, :], qT[hs, tsl],
                                         Kst[hs, b, :], start=True, stop=False)
                    nc.tensor.matmul(ps_ok[:, b, h, :], am_a[:, c, h, b, :],
                                     stil_a[:, c, b, hs],
                                     start=first, stop=True)
            # slot-softmax WITHOUT the denominator: the per-(t,h) positive
            # scale cancels in the downstream RMS norm over dv (eps there
            # is set tiny so the cancellation is exact to fp precision)
            oksc = sesb.tile([128, 4, 128], F32, name="oksc")
            nc.vector.tensor_tensor(
                oksc[:], ps_ok[:].rearrange("p b h f -> p b (h f)"),
                lam_a[:, c], ALU.mult)
            ex = sesb.tile([128, 4, 128], BF16, name="ex")
            nc.scalar.activation(ex[:], oksc[:], AF.Exp, scale=0.125)
            pl = sesb.tile([128, 4, 128], BF16, name="pl")
            nc.vector.tensor_tensor(pl[:], ex[:], lam_a[:, c], ALU.mult)
            ps_pt = seps.tile([128, 4, 128], BF16, name="ps_pt")
            for b in range(4):
                nc.tensor.transpose(ps_pt[:, b, :], pl[:, b, :], ident_sb)
            plT = sesb.tile([128, 4, 128], BF16, name="plT")
            nc.scalar.activation(plT[:], ps_pt[:], AF.Copy)
            ps_o = seps.tile([128, 4, 128], F32, name="ps_o")
            for h in range(2):
                hs = slice(h * 64, (h + 1) * 64)
                ps_b2 = seps.tile([128, 4, 128], F32, name="ps_b2")
                for b in range(4):
                    nc.tensor.matmul(ps_b2[:, b, :], stT_a[hs, c, b, :],
                                     plT[hs, b, :], start=True, stop=True)
                b2m = sesb.tile([128, 4, 128], BF16, name="b2m")
                nc.vector.tensor_tensor(b2m[:], ps_b2[:], cmask_bc, ALU.mult)
                for b in range(4):
                    bi = b * 4 + c
                    if not first:
                        nc.tensor.matmul(ps_o[hs, b, :], Vst[hs, b, :],
                                         plT[hs, b, :], start=True, stop=False)
                    nc.tensor.matmul(ps_o[hs, b, :], v_tm[:, bi, hs],
                                     b2m[:, b, :], start=first, stop=True)
            onv = onT[:].rearrange("p (b c f) -> p c b f", b=4, c=NCH)
            nc.scalar.activation(onv[:, c], ps_o[:], AF.Copy)
            if first:
                nc.vector.tensor_copy(Kst[:], dk_a[:, c])
                nc.vector.tensor_copy(Vst[:], dv_a[:, c])
            else:
                for h in range(2):
                    hs = slice(h * 64, (h + 1) * 64)
                    nc.vector.tensor_tensor(Kst[hs], Kst[hs],
                                            lbc_a[hs, c, :, hs], ALU.mult)
                nc.vector.tensor_tensor(Kst[:], Kst[:], dk_a[:, c], ALU.add)
                nc.vector.tensor_tensor(
                    Vst[:], Vst[:],
                    dec_a[:, c].rearrange("p (b o) -> p b o", b=4)
                    .to_broadcast([128, 4, 64]),
                    ALU.mult)
                nc.vector.tensor_tensor(Vst[:], Vst[:], dv_a[:, c], ALU.add)

    # batched RMS over dv for the whole onT
    with tc.tile_pool(name="rms_ps", bufs=2, space="PSUM") as rps, \
         tc.tile_pool(name="rms_sb", bufs=2) as rsb:
        for q4 in range(4):
            qsl = slice(q4 * 512, (q4 + 1) * 512)
            sqo = rsb.tile([128, 512], BF16, name="sqo")
            nc.vector.tensor_tensor(sqo[:], onT[:, qsl], onT[:, qsl], ALU.mult)
            ps_ss = rps.tile([128, 512], F32, name="ps_ss")
            nc.tensor.matmul(ps_ss[:], bd128_sb, sqo[:], start=True, stop=True)
            srt = rsb.tile([128, 512], F32, name="srt")
            nc.scalar.activation(srt[:], ps_ss[:], AF.Sqrt, bias=eps2_sb[:],
                                 scale=1.0 / DV)
            rro = rsb.tile([128, 512], F32, name="rro")
            nc.vector.reciprocal(rro[:], srt[:])
            nc.vector.tensor_tensor(onT[:, qsl], onT[:, qsl], rro[:], ALU.mult)

    if (d := dump("onT", [128, TOK], BF16)) is not None:
        nc.sync.dma_start(d.ap(), onT[:])

    # head-sharded -> token-sharded redistribution
    nc.sync.dma_start(
        a2a_in[:].rearrange("(r p) t -> p r t", p=P),
        onT[:].rearrange("p (r t) -> p r t", r=N_CORES))
    nc.gpsimd.collective_compute("AllToAll", ALU.bypass, replica_groups=RG,
                                 ins=[a2a_in.opt()], outs=[a2a_out.opt()])
    scank.release()

    # w2 prefetch (ACT queue): double-buffered chunks; first two issued
    # here, last two after MLP1 emission (their WAR deps are MLP2 reads,
    # which must already be behind them in the Act queue to avoid a
    # head-of-line deadlock)
    w2p = tc.alloc_tile_pool(name="w2p", bufs=2)
    w2_chunks = []

    def w2_chunk_dma(q4):
        w2t = w2p.tile([128, 8, D], BF16, name="w2t")
        nc.scalar.dma_start(
            w2t[:], w2.ap().rearrange("(n p) d -> p n d", p=P)
            [:, q4 * 8:(q4 + 1) * 8, :])
        w2_chunks.append(w2t)

    w2_chunk_dma(0)
    w2_chunk_dma(1)

    # =====================================================================
    # P4 tail: out-proj + residual + LN2 + MLP on 256 tokens
    # =====================================================================
    tkb = tc.alloc_tile_pool(name="tail_keep", bufs=1)
    ofT = tkb.tile([128, DT, TAIL], BF16, name="ofT")
    nc.sync.dma_start(ofT[:], a2a_out[:].rearrange("(jt p) t -> p jt t", p=P))
    h2 = tkb.tile([128, 2, D], BF16, name="h2")
    h2T = tkb.tile([128, DT, TAIL], BF16, name="h2T")
    zT = tkb.tile([128, MT, TAIL], BF16, name="zT")
    ys = tkb.tile([128, 2, D], F32, name="ys")

    with tc.tile_pool(name="op_ps", bufs=2, space="PSUM") as ops, \
         tc.tile_pool(name="tail_sb", bufs=2) as tsb:
        for tt2 in range(2):
            for nb in range(2):
                nsl = slice(nb * 512, (nb + 1) * 512)
                op_bank = ops.tile([128, 512], F32, name="op_bank")
                for jt in range(DT):
                    nc.tensor.matmul(op_bank[:],
                                     ofT[:, jt, tt2 * 128:(tt2 + 1) * 128],
                                     wo_sb[:, jt, nsl],
                                     start=(jt == 0), stop=(jt == DT - 1))
                nc.vector.tensor_tensor(x2_sb[:, tt2, nsl], op_bank[:],
                                        x2_sb[:, tt2, nsl], ALU.add)
        if (d := dump("x2", [128, 2 * D])) is not None:
            nc.sync.dma_start(d.ap().rearrange("p (n f) -> p n f", n=2),
                              x2_sb[:])

        # LN2 (token-major) -> h2 -> h2T via PE transposes
        ssum = tsb.tile([128, 2], F32, name="ssum")
        ssq = tsb.tile([128, 2], F32, name="ssq")
        sqd = tsb.tile([128, D], BF16, name="sqd")
        for tt2 in range(2):
            nc.vector.tensor_reduce(ssum[:, tt2:tt2 + 1], x2_sb[:, tt2, :],
                                    AX.X, ALU.add)
            nc.scalar.activation(sqd[:], x2_sb[:, tt2, :], AF.Square,
                                 accum_out=ssq[:, tt2:tt2 + 1])
        mu2 = tsb.tile([128, 2], F32, name="mu2")
        nc.vector.tensor_scalar_mul(mu2[:], ssum[:], 1.0 / D)
        var2 = tsb.tile([128, 2], F32, name="var2")
        nc.vector.tensor_tensor(var2[:], mu2[:], mu2[:], ALU.mult)
        ex22 = tsb.tile([128, 2], F32, name="ex22")
        nc.vector.tensor_scalar_mul(ex22[:], ssq[:], 1.0 / D)
        nc.vector.tensor_tensor(var2[:], ex22[:], var2[:], ALU.subtract)
        sd2 = tsb.tile([128, 2], F32, name="sd2")
        nc.scalar.activation(sd2[:], var2[:], AF.Sqrt, bias=eps_sb[:])
        r2 = tsb.tile([128, 2], F32, name="r2")
        nc.vector.reciprocal(r2[:], sd2[:])
        nmu2 = tsb.tile([128, 2], F32, name="nmu2")
        nc.vector.tensor_tensor(nmu2[:], r2[:], mu2[:], ALU.mult)
        nc.vector.tensor_scalar_mul(nmu2[:], nmu2[:], -1.0)
        for tt2 in range(2):
            nc.scalar.activation(h2[:, tt2, :], x2_sb[:, tt2, :], AF.Identity,
                                 bias=nmu2[:, tt2:tt2 + 1],
                                 scale=r2[:, tt2:tt2 + 1])

    with tc.tile_pool(name="h2t_ps", bufs=2, space="PSUM") as hps:
        for dt in range(DT):
            ph = hps.tile([128, 2, 128], BF16, name="ph")
            for tt2 in range(2):
                nc.tensor.transpose(ph[:, tt2, :],
                                    h2[:, tt2, dt * 128:(dt + 1) * 128],
                                    ident_sb)
            nc.scalar.activation(h2T[:, dt, :], ph[:], AF.Copy)

    # MLP1 m-major: y1T[mt] = sum_dt w1[dt,mt]^T h2T[dt]; gelu -> zT
    # (b1 + ln2_b@W1 is all-zero for this problem -- asserted on host)
    with tc.tile_pool(name="y1_ps", bufs=3, space="PSUM") as y1ps:
        for m2 in range(MT // 2):
            y1b = y1ps.tile([128, 2, TAIL], F32, name="y1b")
            for half in range(2):
                mt = m2 * 2 + half
                msl = slice(mt * 128, (mt + 1) * 128)
                for dt in range(DT):
                    nc.tensor.matmul(y1b[:, half, :], w1_sb[:, dt, msl],
                                     h2T[:, dt, :],
                                     start=(dt == 0), stop=(dt == DT - 1))
            nc.scalar.activation(zT[:, m2 * 2:(m2 + 1) * 2, :], y1b[:],
                                 AF.Gelu)

    w2_chunk_dma(2)
    w2_chunk_dma(3)

    # MLP2: y2[t,:] = sum_mt zT[mt]^T w2[mt]
    with tc.tile_pool(name="y2_ps", bufs=1, space="PSUM") as y2ps:
        y2_banks = [y2ps.tile([128, 512], F32, name=f"y2b{i}")
                    for i in range(4)]
        for mt in range(MT):
            w2t = w2_chunks[mt // 8]
            for tt2 in range(2):
                for nb in range(2):
                    nc.tensor.matmul(
                        y2_banks[tt2 * 2 + nb],
                        zT[:, mt, tt2 * 128:(tt2 + 1) * 128],
                        w2t[:, mt % 8, nb * 512:(nb + 1) * 512],
                        start=(mt == 0), stop=(mt == MT - 1))
        for tt2 in range(2):
            for nb in range(2):
                nsl = slice(nb * 512, (nb + 1) * 512)
                nc.vector.tensor_tensor(ys[:, tt2, nsl],
                                        y2_banks[tt2 * 2 + nb],
                                        x2_sb[:, tt2, nsl], ALU.add)
    nc.sync.dma_start(y_out.ap().rearrange("(n p) d -> p n d", p=P), ys[:])

    for pool in (tkb, w2p, w1p, persist, dram, const):
        pool.release()


def _build():
    nc = bacc.Bacc("TRN2", target_bir_lowering=False, debug=False,
                   num_devices=N_CORES)

    def din(name, shape, dt=BF16):
        return nc.dram_tensor(name, shape, dt, kind="ExternalInput")

    io = dict(
        xT=din("xT", [D, TOK]),
        x_res=din("x_res", [TAIL, D], F32),
        wqkvf=din("wqkvf", [D, 512]),
        wo=din("wo", [D, D]),
        w1=din("w1", [D, MLP]),
        w2=din("w2", [MLP, D]),
        bcols=din("bcols", [128, 4], F32),
        blob_f32=din("blob_f32", [128, 129], F32),
        blob_bf16=din("blob_bf16", [128, 3 * 128]),
        rowblob=din("rowblob", [1, 10 * 128]),
        y_out=nc.dram_tensor("y_out", [TAIL, D], F32, kind="ExternalOutput"),
    )

    dbg = [s for s in os.environ.get("GSA_DEBUG", "").split(",") if s]
    dbg_outs = {}

    def dump(name, shape, dt=F32):
        if name in dbg:
            t = nc.dram_tensor("dbg_" + name, shape, dt,
                               kind="ExternalOutput")
            dbg_outs[name] = t
            return t
        return None

    io["dump"] = dump
    with tile.TileContext(nc) as tcx:
        _emit(nc, tcx, io)
    nc.compile()
    return nc, sorted(dbg_outs)


def _host_prep(inputs):
    """Fold norms/biases into weights; build per-core in_maps."""
    f32 = np.float32
    bf16 = ml_dtypes.bfloat16
    x = np.asarray(inputs["hidden_states"], f32).reshape(TOK, D)
    ln1_w = np.asarray(inputs["ln1_w"], f32)
    ln1_b = np.asarray(inputs["ln1_b"], f32)
    ln2_w = np.asarray(inputs["ln2_w"], f32)
    ln2_b = np.asarray(inputs["ln2_b"], f32)
    gnorm = np.asarray(inputs["gnorm_w"], f32)
    Wq = np.asarray(inputs["Wq"], f32) * ln1_w[:, None]
    Wk = np.asarray(inputs["Wk"], f32) * ln1_w[:, None]
    Wv = np.asarray(inputs["Wv"], f32) * ln1_w[:, None]
    Wf = np.asarray(inputs["Wf"], f32) * ln1_w[:, None]
    bq = ln1_b @ np.asarray(inputs["Wq"], f32)
    bk = ln1_b @ np.asarray(inputs["Wk"], f32)
    bv = ln1_b @ np.asarray(inputs["Wv"], f32)
    bf_ = ln1_b @ np.asarray(inputs["Wf"], f32)
    assert np.allclose(bv, 0) and np.allclose(bf_, 0), \
        "v/f projection biases must be zero (ln1_b fold only done for q/k)"
    Wo = np.asarray(inputs["Wo"], f32) * np.tile(gnorm, H)[:, None]
    W1 = np.asarray(inputs["W1"], f32) * ln2_w[:, None]
    b1 = np.asarray(inputs["b1"], f32) + ln2_b @ np.asarray(inputs["W1"], f32)
    assert np.allclose(b1, 0), "MLP1 bias must be zero (not emitted on device)"
    W2 = np.asarray(inputs["W2"], f32)
    b2 = np.asarray(inputs["b2"], f32)

    tri = np.tril(np.ones((128, 128), f32))  # [t, tau] tau<=t
    xT = np.ascontiguousarray(x.T.astype(bf16))

    common = dict(
        xT=xT,
        wo=np.ascontiguousarray(Wo.astype(bf16)),
        w1=np.ascontiguousarray(W1.astype(bf16)),
        w2=np.ascontiguousarray(W2.astype(bf16)),
        blob_f32=np.ascontiguousarray(np.concatenate(
            [(-0.125 * tri).T, np.ones((128, 1), f32)], axis=1)),
        blob_bf16=np.ascontiguousarray(np.concatenate(
            [tri.T, np.eye(128, dtype=f32),
             np.kron(np.eye(2, dtype=f32), np.ones((64, 64), f32))],
            axis=1).astype(bf16)),
    )
    in_maps = []
    for r in range(N_CORES):
        jsl = slice(r * 128, (r + 1) * 128)  # 2 heads = 128 cols
        m = dict(common)
        m["x_res"] = np.ascontiguousarray(x[r * TAIL:(r + 1) * TAIL]
                                          + b2[None, :])
        wq, wk = Wq[:, jsl], Wk[:, jsl]
        wv, wf = Wv[:, jsl], Wf[:, jsl]
        m["wqkvf"] = np.ascontiguousarray(
            np.concatenate([wq, wk, wv, wf], axis=1).astype(bf16))
        rb = np.zeros((1, 10 * 128), f32)
        for i, w in enumerate((wq, wk, wv, wf)):
            rb[0, i * 128:(i + 1) * 128] = -w.sum(axis=0)
        rb[0, 8 * 128:9 * 128] = 1.0
        rb[0, 9 * 128] = 1.0
        m["rowblob"] = np.ascontiguousarray(rb.astype(bf16))
        bc = np.zeros((128, 4), f32)
        bc[:, 0], bc[:, 1] = bq[jsl], bk[jsl]
        m["bcols"] = np.ascontiguousarray(bc)
        in_maps.append(m)
    return in_maps


def kernel(**inputs):
    if "nc" not in _cache:
        _cache["nc"], _cache["dbg"] = _build()
    nc = _cache["nc"]
    in_maps = _host_prep(inputs)
    res = run_bass_kernel_spmd(nc, in_maps, core_ids=list(range(N_CORES)),
                               trace=bool(os.environ.get("GSA_TRACE")))
    _cache["last_results"] = res
    out = np.concatenate([res.results[r]["y_out"] for r in range(N_CORES)],
                         axis=0)
    return out.reshape(B, T, D)


# revision 38
# speedup vs baseline: 1.1287x; 1.1287x over previous
"""GSA video block kernel for 8 TRN2 NeuronCores (v2).

Sharding: head-parallel attention (2 heads/core) -> one AllToAll that
redistributes the RMS-normed head outputs from head-sharded to
token-sharded -> token-parallel tail (out-proj + LN2 + MLP with full
weights, 256 tokens/core).

v2 layout strategy: the host supplies x TRANSPOSED (xT [D, TOK]), so
LN1 never materializes h -- projections run feat-major from xT with a
rank-1 mean correction accumulated into the psum and a broadcast-rstd
multiply afterwards.  Token-major k/v/f come from tensor-engine
transposes (no DMA transposes anywhere).  The chunked scan processes
the 4 batches' chunk-c tiles as single [128,512]-wide ops, with all
state-independent prep emitted ahead of the serial state chain.  The
MLP computes y1 m-major so MLP2 needs no transpose.
"""

import os
import sys

import numpy as np
import ml_dtypes

if "/opt/trn_rl_repo" not in sys.path:
    sys.path.insert(0, "/opt/trn_rl_repo")

import concourse.bass as bass  # noqa: E402
import concourse.mybir as mybir  # noqa: E402
import concourse.tile as tile  # noqa: E402
from concourse import bacc  # noqa: E402
from concourse.bass_utils import run_bass_kernel_spmd  # noqa: E402

BF16 = mybir.dt.bfloat16
F32 = mybir.dt.float32
AF = mybir.ActivationFunctionType
ALU = mybir.AluOpType
AX = mybir.AxisListType

B, T, D = 4, 512, 1024
H, DK, DV, M = 16, 64, 64, 64
MLP = 4096
EPS = 1e-6

N_CORES = 8
C = 128                    # scan chunk length
NCH = T // C               # chunks per batch = 4
TOK = B * T                # 2048 flat tokens
TT = TOK // 128            # 16 token tiles
DT = D // 128              # 8 d tiles
MT = MLP // 128            # 32 mlp tiles
TAIL = TOK // N_CORES      # 256 tokens per core in the tail
RG = [list(range(N_CORES))]
P = 128

_cache = {}


def _emit(nc, tc, io):
    xT, x_res = io["xT"], io["x_res"]
    wqkvf, wo, w1, w2 = io["wqkvf"], io["wo"], io["w1"], io["w2"]
    blob_f32, blob_bf16, rowblob = io["blob_f32"], io["blob_bf16"], io["rowblob"]
    y_out, dump = io["y_out"], io["dump"]

    const = tc.alloc_tile_pool(name="const", bufs=1)
    dram = tc.alloc_tile_pool(name="dram", bufs=1, space="DRAM")

    # ---- warmup collective (prepay ncfw handshake) -----------------------
    wa_in = dram.tile([8, 128], BF16, name="wa_in")
    wa_out = dram.tile([8, 128], BF16, name="wa_out")
    nc.gpsimd.collective_compute("AllReduce", ALU.add, replica_groups=RG,
                                 ins=[wa_in.opt()], outs=[wa_out.opt()])
    wa2_in = dram.tile([8, 1024], BF16, name="wa2_in")
    wa2_out = dram.tile([8, 1024], BF16, name="wa2_out")
    nc.gpsimd.collective_compute("AllToAll", ALU.bypass, replica_groups=RG,
                                 ins=[wa2_in.opt()], outs=[wa2_out.opt()])

    # split A2A: half 1 ships chunk-parity-0 tokens (each dest core's first
    # 128 tail tokens), half 2 the parity-1 tokens
    a2a_ins = [dram.tile([128 * N_CORES, 128], BF16, name=f"a2a_in{i}")
               for i in range(2)]
    a2a_outs = [dram.tile([128 * N_CORES, 128], BF16, name=f"a2a_out{i}")
                for i in range(2)]

    # ---- constants into SBUF (SP queue) ---------------------------------
    cf = const.tile([128, 129], F32, name="cf")
    nc.sync.dma_start(cf[:], blob_f32.ap())
    ltriT_sb = cf[:, 0:128]
    onescol = cf[:, 128:129]                              # all ones
    cb = const.tile([128, 3, 128], BF16, name="cb")       # cmask|ident|bd128
    nc.sync.dma_start(cb[:], blob_bf16.ap().rearrange("p (n f) -> p n f", n=3))
    cmask_bc = cb[:, 0:1, :].to_broadcast([128, 4, 128])
    ident_sb, bd128_sb = cb[:, 1, :], cb[:, 2, :]
    rows = const.tile([1, 10, 128], BF16, name="rows")
    nc.sync.dma_start(rows[:], rowblob.ap().rearrange("o (n f) -> o n f", n=10))
    csq_r, csk_r, csv_r, csf_r = (rows[:, i, :] for i in range(4))
    ones_row = rows[:, 8, :]
    one1 = rows[:, 9, 0:1]
    bcol = const.tile([128, 4], F32, name="bcol")
    nc.sync.dma_start(bcol[:], io["bcols"].ap())
    eps_sb = const.tile([128, 1], F32)
    nc.vector.memset(eps_sb[:], EPS)
    eps2_sb = const.tile([128, 1], F32)
    nc.vector.memset(eps2_sb[:], 1e-30)

    # ---- persistent activation tensors (allocated below xp/wp in the
    # pool stack; released only at the very end) --------------------------
    persist = tc.alloc_tile_pool(name="persist", bufs=1)
    qT = persist.tile([128, TOK], BF16, name="qT")
    kT = persist.tile([128, TOK], BF16, name="kT")
    k_tm = persist.tile([128, TT, 128], BF16, name="k_tm")
    v_tm = persist.tile([128, TT, 128], BF16, name="v_tm")
    sp = persist.tile([128, TT, 128], F32, name="sp")
    s_tm = persist.tile([128, TT, 128], BF16, name="s_tm")
    onT = persist.tile([128, TOK], BF16, name="onT")

    # ---- bulk loads -----------------------------------------------------
    xp = tc.alloc_tile_pool(name="xp", bufs=1)
    xT_sb = xp.tile([128, DT, TOK], BF16, name="xT_sb")
    for dt in range(DT):
        nc.sync.dma_start(xT_sb[:, dt, :],
                          xT.ap().rearrange("(dt p) t -> p dt t", p=P)[:, dt, :])
    wp = tc.alloc_tile_pool(name="wp", bufs=1)
    w4_sb = wp.tile([128, DT, 512], BF16, name="w4_sb")   # wq|wk|wv|wf cols
    nc.scalar.dma_start(w4_sb[:], wqkvf.ap().rearrange("(dt p) j -> p dt j", p=P))
    wo_sb = const.tile([128, DT, D], BF16, name="wo_sb")
    nc.scalar.dma_start(wo_sb[:], wo.ap().rearrange("(dt p) j -> p dt j", p=P))
    x2_sb = const.tile([128, 2, D], F32, name="x2_sb")
    nc.sync.dma_start(x2_sb[:], x_res.ap().rearrange("(n p) d -> p n d", p=P))

    # =====================================================================
    # P1: LN1 stats from xT via matmul reduction (per 512-token group)
    # =====================================================================
    stats = tc.alloc_tile_pool(name="stats", bufs=1)
    mu_r = stats.tile([1, 4, 512], BF16, name="mu_r")
    sd_r = stats.tile([1, 4, 512], BF16, name="sd_r")
    rbc = stats.tile([128, 4, 512], F32, name="rbc")
    oc = stats.tile([128, 1], BF16, name="oc")
    nc.vector.memset(oc[:], 1.0 / D)
    with tc.tile_pool(name="sq_p", bufs=3) as sqp, \
         tc.tile_pool(name="st_ps", bufs=2, space="PSUM") as stps, \
         tc.tile_pool(name="rb_ps", bufs=2, space="PSUM") as rbps, \
         tc.tile_pool(name="st_sb", bufs=2) as stsb:
        for g in range(4):
            gsl = slice(g * 512, (g + 1) * 512)
            ps_mu = stps.tile([1, 512], F32, name="ps_mu")
            ps_e2 = stps.tile([1, 512], F32, name="ps_e2")
            for dt in range(DT):
                sq = sqp.tile([128, 512], BF16, name="sq")
                nc.vector.tensor_tensor(sq[:], xT_sb[:, dt, gsl],
                                        xT_sb[:, dt, gsl], ALU.mult)
                nc.tensor.matmul(ps_mu[:], oc[:], xT_sb[:, dt, gsl],
                                 start=(dt == 0), stop=(dt == DT - 1))
                nc.tensor.matmul(ps_e2[:], oc[:], sq[:],
                                 start=(dt == 0), stop=(dt == DT - 1))
            # var = e2 - mu^2 ; sd = sqrt(var+eps); rbc = 1/bcast(sd)
            nc.scalar.activation(mu_r[:, g, :], ps_mu[:], AF.Copy)
            var = stsb.tile([1, 512], F32, name="var")
            nc.vector.tensor_tensor(var[:], mu_r[:, g, :], mu_r[:, g, :],
                                    ALU.mult)
            nc.vector.tensor_tensor(var[:], ps_e2[:], var[:], ALU.subtract)
            nc.scalar.activation(sd_r[:, g, :], var[:], AF.Sqrt,
                                 bias=eps_sb[0:1, :])
            ps_rb = rbps.tile([128, 512], F32, name="ps_rb")
            nc.tensor.matmul(ps_rb[:], ones_row, sd_r[:, g, :],
                             start=True, stop=True)
            nc.vector.reciprocal_approx_fast(rbc[:, g, :], ps_rb[:])

    # =====================================================================
    # P2: projections, feat-major, with rank-1 mean correction
    # =====================================================================
    ffeat = tc.alloc_tile_pool(name="ffeat", bufs=1)
    f_ft = ffeat.tile([128, TOK], BF16, name="f_ft")
    with tc.tile_pool(name="pj_ps", bufs=3, space="PSUM") as pjps, \
         tc.tile_pool(name="pj_sb", bufs=3) as pjsb, \
         tc.tile_pool(name="tr_ps", bufs=2, space="PSUM") as trps:
        def proj_psum(jsl, cs_row, g):
            gsl = slice(g * 512, (g + 1) * 512)
            bank = pjps.tile([128, 512], F32, name="pjbank")
            for dt in range(DT):
                nc.tensor.matmul(bank[:], w4_sb[:, dt, jsl], xT_sb[:, dt, gsl],
                                 start=(dt == 0), stop=False)
            nc.tensor.matmul(bank[:], cs_row, mu_r[:, g, :],
                             start=False, stop=True)
            return bank

        for g in range(4):                      # q: silu -> qT
            bank = proj_psum(slice(0, 128), csq_r, g)
            gsl = slice(g * 512, (g + 1) * 512)
            yp = pjsb.tile([128, 512], BF16, name="yp")
            nc.vector.tensor_tensor(yp[:], bank[:], rbc[:, g, :], ALU.mult)
            nc.scalar.activation(qT[:, gsl], yp[:], AF.Silu,
                                 bias=bcol[:, 0:1])
        for g in range(4):                      # k: silu -> kT (+ k_tm below)
            bank = proj_psum(slice(128, 256), csk_r, g)
            gsl = slice(g * 512, (g + 1) * 512)
            yp = pjsb.tile([128, 512], BF16, name="yp")
            nc.vector.tensor_tensor(yp[:], bank[:], rbc[:, g, :], ALU.mult)
            nc.scalar.activation(kT[:, gsl], yp[:], AF.Silu,
                                 bias=bcol[:, 1:2])
        for g in range(4):
            pst = trps.tile([128, 4, 128], BF16, name="pst")
            for cc in range(4):
                ti = g * 4 + cc
                nc.tensor.transpose(pst[:, cc, :],
                                    kT[:, ti * 128:(ti + 1) * 128], ident_sb)
            nc.vector.tensor_copy(k_tm[:, g * 4:(g + 1) * 4, :], pst[:])
        for g in range(4):                      # v: mult only -> v_tm
            bank = proj_psum(slice(256, 384), csv_r, g)
            yp = pjsb.tile([128, 512], BF16, name="yp")
            nc.vector.tensor_tensor(yp[:], bank[:], rbc[:, g, :], ALU.mult)
            pst = trps.tile([128, 4, 128], BF16, name="pst")
            for cc in range(4):
                nc.tensor.transpose(pst[:, cc, :], yp[:, cc * 128:(cc + 1) * 128],
                                    ident_sb)
            nc.vector.tensor_copy(v_tm[:, g * 4:(g + 1) * 4, :], pst[:])
        for g in range(4):                      # f -> f_ft (feat-major)
            bank = proj_psum(slice(384, 512), csf_r, g)
            gsl = slice(g * 512, (g + 1) * 512)
            nc.vector.tensor_tensor(f_ft[:, gsl], bank[:], rbc[:, g, :],
                                    ALU.mult)
        # gates: sp = softplus(-f) = ln(1+exp(-f)); three passes so the
        # activation table switches exp -> ln -> exp only once each
        enf_a = pjsb.tile([128, TT, 128], F32, name="enf_a", bufs=1)
        for g in range(4):
            pst = trps.tile([128, 4, 128], BF16, name="pst")
            for cc in range(4):
                ti = g * 4 + cc
                nc.tensor.transpose(pst[:, cc, :],
                                    f_ft[:, ti * 128:(ti + 1) * 128], ident_sb)
            csl = slice(g * 4, (g + 1) * 4)
            nc.scalar.activation(enf_a[:, csl, :], pst[:], AF.Exp, scale=-1.0)
        for g in range(4):
            csl = slice(g * 4, (g + 1) * 4)
            nc.scalar.activation(sp[:, csl, :], enf_a[:, csl, :], AF.Ln,
                                 bias=1.0)
        for g in range(4):
            csl = slice(g * 4, (g + 1) * 4)
            e8 = pjsb.tile([128, 4, 128], BF16, name="e8")
            nc.scalar.activation(e8[:], sp[:, csl, :], AF.Exp, scale=-0.125)
            nc.vector.tensor_scalar(s_tm[:, csl, :], e8[:], -1.0, 1.0,
                                    ALU.mult, ALU.add)
    ffeat.release()
    stats.release()
    wp.release()
    xp.release()

    # start w1 prefetch now (SYNC queue -- keeps the scalar queue free for
    # the scan's activations), in mlp-dim chunks
    w1p = tc.alloc_tile_pool(name="w1p", bufs=1)
    w1_sb = w1p.tile([128, DT, MLP], BF16, name="w1_sb")
    for q4 in range(4):
        msl = slice(q4 * 1024, (q4 + 1) * 1024)
        nc.sync.dma_start(
            w1_sb[:, :, msl],
            w1.ap().rearrange("(dt p) m -> p dt m", p=P)[:, :, msl])

    for nm, t_sb in (("qT", qT), ("kT", kT)):
        if (d := dump(nm, [128, TOK], BF16)) is not None:
            nc.sync.dma_start(d.ap(), t_sb[:])
    for nm, t_sb in (("k_tm", k_tm), ("v_tm", v_tm), ("s_tm", s_tm)):
        if (d := dump(nm, [128, TT * 128], BF16)) is not None:
            nc.sync.dma_start(d.ap().rearrange("p (n f) -> p n f", n=TT), t_sb[:])
    if (d := dump("sp", [128, TT * 128])) is not None:
        nc.sync.dma_start(d.ap().rearrange("p (n f) -> p n f", n=TT), sp[:])

    # =====================================================================
    # P3: chunked scan.  Token tile index = b*4+c.  Group = fixed c, 4 b's.
    # =====================================================================
    scank = tc.alloc_tile_pool(name="scank", bufs=1)
    Kst = scank.tile([128, 4, 64], BF16, name="Kst")     # [(2h dk), b, m]
    Vst = scank.tile([128, 4, 64], BF16, name="Vst")     # [(2h m), b, dv]
    lam_a = scank.tile([128, NCH, 4, 128], BF16, name="lam_a")
    stil_a = scank.tile([128, NCH, 4, 128], BF16, name="stil_a")
    stT_a = scank.tile([128, NCH, 4, 128], BF16, name="stT_a")
    am_a = scank.tile([128, NCH, 2, 4, 128], BF16, name="am_a")
    dk_a = scank.tile([128, NCH, 4, 64], BF16, name="dk_a")   # [(2h dk), b, m]
    dv_a = scank.tile([128, NCH, 4, 64], BF16, name="dv_a")   # [(2h m), b, dv]
    lbc_a = scank.tile([128, NCH, 4, 128], BF16, name="lbc_a")
    dec_a = scank.tile([128, NCH, 4], BF16, name="dec_a")

    def cgv(t_sb, c):
        # [128, TT, f] -> [128, 4b, f] strided view for chunk c
        return t_sb[:].rearrange("p (b c) f -> p c b f", c=NCH)[:, c]

    # serial-phase + rms psum pools FIRST so their banks are disjoint from
    # the prep rotation's
    with tc.tile_pool(name="se_ps", bufs=1, space="PSUM") as seps, \
         tc.tile_pool(name="rms_ps", bufs=2, space="PSUM") as rps, \
         tc.tile_pool(name="pr_ps", bufs=2, space="PSUM") as prps, \
         tc.tile_pool(name="pr_sb", bufs=2) as prsb, \
         tc.tile_pool(name="rms_sb", bufs=2) as rsb, \
         tc.tile_pool(name="se_sb", bufs=2) as sesb:

        def rms_and_ship(par):
            # normalize + pack + AllToAll for tiles of chunk parity `par`
            onh = onT[:].rearrange("p (x e f) -> p e x f", x=8, e=2)[:, par]
            for q2 in range(2):
                qv = onh[:, q2 * 4:(q2 + 1) * 4, :]
                sqo = rsb.tile([128, 4, 128], BF16, name="sqo")
                nc.vector.tensor_tensor(sqo[:], qv, qv, ALU.mult)
                ps_ss = rps.tile([128, 512], F32, name="ps_ss")
                nc.tensor.matmul(ps_ss[:], bd128_sb, sqo[:],
                                 start=True, stop=True)
                srt = rsb.tile([128, 512], F32, name="srt")
                nc.scalar.activation(srt[:], ps_ss[:], AF.Sqrt,
                                     bias=eps2_sb[:], scale=1.0 / DV)
                rro = rsb.tile([128, 4, 128], F32, name="rro")
                nc.vector.reciprocal_approx_fast(
                    rro[:].rearrange("p b f -> p (b f)"), srt[:])
                nc.vector.tensor_tensor(qv, qv, rro[:], ALU.mult)
            nc.sync.dma_start(
                a2a_ins[par][:].rearrange("(r p) t -> p r t", p=P), onh)
            nc.gpsimd.collective_compute(
                "AllToAll", ALU.bypass, replica_groups=RG,
                ins=[a2a_ins[par].opt()], outs=[a2a_outs[par].opt()])

        # ---------------- prep (state-independent), all c ----------------
        for c in range(NCH):
            ps_cs = prps.tile([128, 4, 128], F32, name="ps_cs", tag="pband")
            nc.tensor.matmul(ps_cs[:], ltriT_sb[:], cgv(sp, c),
                             start=True, stop=True)
            nc.scalar.activation(lam_a[:, c], ps_cs[:], AF.Exp)
            en4 = prsb.tile([128, 4, 128], BF16, name="en4")
            nc.scalar.activation(en4[:], ps_cs[:], AF.Exp, scale=-1.0)
            ps_ct = prps.tile([1, 4, 128], F32, name="ps_ct", tag="pband")
            nc.tensor.matmul(ps_ct[:], onescol, cgv(sp, c),
                             start=True, stop=True)
            lamCr = prsb.tile([1, 4, 128], BF16, name="lamCr")
            nc.scalar.activation(lamCr[:], ps_ct[:], AF.Exp, scale=-0.125)
            ps_lb = prps.tile([128, 4, 128], F32, name="ps_lb", tag="pband")
            nc.tensor.matmul(ps_lb[:], ones_row,
                             lamCr[:].rearrange("o b f -> o (b f)"),
                             start=True, stop=True)
            nc.scalar.activation(lbc_a[:, c], ps_lb[:], AF.Copy)
            nc.vector.tensor_tensor(stil_a[:, c], cgv(s_tm, c), en4[:],
                                    ALU.mult)
            s2 = prsb.tile([128, 4, 128], BF16, name="s2")
            nc.vector.tensor_tensor(s2[:], stil_a[:, c], ps_lb[:], ALU.mult)
            ps_dc = prps.tile([128, 4], F32, name="ps_dc", tag="pband")
            for b in range(4):
                nc.tensor.matmul(ps_dc[:, b:b + 1], lamCr[:, b, :], one1,
                                 start=True, stop=True)
            nc.scalar.activation(dec_a[:, c], ps_dc[:], AF.Copy)
            for h in range(2):
                hs = slice(h * 64, (h + 1) * 64)
                ps_a = prps.tile([128, 4, 128], F32, name="ps_a", tag="pband")
                for b in range(4):
                    tsl = slice((b * 4 + c) * 128, (b * 4 + c + 1) * 128)
                    nc.tensor.matmul(ps_a[:, b, :], kT[hs, tsl], qT[hs, tsl],
                                     start=True, stop=True)
                nc.vector.tensor_tensor(am_a[:, c, h], ps_a[:], cmask_bc,
                                        ALU.mult)
            ps_st = prps.tile([128, 4, 128], BF16, name="ps_st", tag="pband")
            for b in range(4):
                nc.tensor.transpose(ps_st[:, b, :], stil_a[:, c, b, :],
                                    ident_sb)
            nc.scalar.activation(stT_a[:, c], ps_st[:], AF.Copy)
            ps_dk = prps.tile([128, 4, 64], F32, name="ps_dk", tag="pband")
            for h in range(2):
                hs = slice(h * 64, (h + 1) * 64)
                for b in range(4):
                    bi = b * 4 + c
                    nc.tensor.matmul(ps_dk[hs, b, :], k_tm[:, bi, hs],
                                     s2[:, b, hs], start=True, stop=True)
            nc.scalar.activation(dk_a[:, c], ps_dk[:], AF.Copy)
            ps_dv = prps.tile([128, 4, 64], F32, name="ps_dv", tag="pband")
            for h in range(2):
                hs = slice(h * 64, (h + 1) * 64)
                for b in range(4):
                    bi = b * 4 + c
                    nc.tensor.matmul(ps_dv[hs, b, :], s2[:, b, hs],
                                     v_tm[:, bi, hs], start=True, stop=True)
            nc.scalar.activation(dv_a[:, c], ps_dv[:], AF.Copy)

        # ---------------- serial state chain -----------------------------
        for c in range(NCH):
            first = (c == 0)
            ps_ok = seps.tile([128, 4, 2, 64], F32, name="ps_ok")
            for h in range(2):
                hs = slice(h * 64, (h + 1) * 64)
                for b in range(4):
                    tsl = slice((b * 4 + c) * 128, (b * 4 + c + 1) * 128)
                    if not first:
                        nc.tensor.matmul(ps_ok[:, b, h, :], qT[hs, tsl],
                                         Kst[hs, b, :], start=True, stop=False)
                    nc.tensor.matmul(ps_ok[:, b, h, :], am_a[:, c, h, b, :],
                                     stil_a[:, c, b, hs],
                                     start=first, stop=True)
            # slot-softmax WITHOUT the denominator: the per-(t,h) positive
            # scale cancels in the downstream RMS norm over dv (eps there
            # is set tiny so the cancellation is exact to fp precision)
            oksc = sesb.tile([128, 4, 128], F32, name="oksc")
            nc.vector.tensor_tensor(
                oksc[:], ps_ok[:].rearrange("p b h f -> p b (h f)"),
                lam_a[:, c], ALU.mult)
            ex = sesb.tile([128, 4, 128], BF16, name="ex")
            nc.scalar.activation(ex[:], oksc[:], AF.Exp, scale=0.125)
            pl = sesb.tile([128, 4, 128], BF16, name="pl")
            nc.vector.tensor_tensor(pl[:], ex[:], lam_a[:, c], ALU.mult)
            ps_pt = seps.tile([128, 4, 128], BF16, name="ps_pt")
            for b in range(4):
                nc.tensor.transpose(ps_pt[:, b, :], pl[:, b, :], ident_sb)
            plT = sesb.tile([128, 4, 128], BF16, name="plT")
            nc.scalar.activation(plT[:], ps_pt[:], AF.Copy)
            ps_o = seps.tile([128, 4, 128], F32, name="ps_o")
            for h in range(2):
                hs = slice(h * 64, (h + 1) * 64)
                ps_b2 = seps.tile([128, 4, 128], F32, name="ps_b2")
                for b in range(4):
                    nc.tensor.matmul(ps_b2[:, b, :], stT_a[hs, c, b, :],
                                     plT[hs, b, :], start=True, stop=True)
                b2m = sesb.tile([128, 4, 128], BF16, name="b2m")
                nc.vector.tensor_tensor(b2m[:], ps_b2[:], cmask_bc, ALU.mult)
                for b in range(4):
                    bi = b * 4 + c
                    if not first:
                        nc.tensor.matmul(ps_o[hs, b, :], Vst[hs, b, :],
                                         plT[hs, b, :], start=True, stop=False)
                    nc.tensor.matmul(ps_o[hs, b, :], v_tm[:, bi, hs],
                                     b2m[:, b, :], start=first, stop=True)
            onv = onT[:].rearrange("p (b c f) -> p c b f", b=4, c=NCH)
            nc.scalar.activation(onv[:, c], ps_o[:], AF.Copy)
            if first:
                nc.vector.tensor_copy(Kst[:], dk_a[:, c])
                nc.vector.tensor_copy(Vst[:], dv_a[:, c])
            else:
                for h in range(2):
                    hs = slice(h * 64, (h + 1) * 64)
                    nc.vector.tensor_tensor(Kst[hs], Kst[hs],
                                            lbc_a[hs, c, :, hs], ALU.mult)
                nc.vector.tensor_tensor(Kst[:], Kst[:], dk_a[:, c], ALU.add)
                nc.vector.tensor_tensor(
                    Vst[:], Vst[:],
                    dec_a[:, c].rearrange("p (b o) -> p b o", b=4)
                    .to_broadcast([128, 4, 64]),
                    ALU.mult)
                nc.vector.tensor_tensor(Vst[:], Vst[:], dv_a[:, c], ALU.add)
            if c == 2:
                rms_and_ship(0)
        rms_and_ship(1)

    if (d := dump("onT", [128, TOK], BF16)) is not None:
        nc.sync.dma_start(d.ap(), onT[:])
    scank.release()

    # w2 prefetch (ACT queue): double-buffered chunks; first two issued
    # here, last two after MLP1 emission (their WAR deps are MLP2 reads,
    # which must already be behind them in the Act queue to avoid a
    # head-of-line deadlock)
    w2p = tc.alloc_tile_pool(name="w2p", bufs=2)
    w2_chunks = []

    def w2_chunk_dma(q4):
        w2t = w2p.tile([128, 8, D], BF16, name="w2t")
        nc.scalar.dma_start(
            w2t[:], w2.ap().rearrange("(n p) d -> p n d", p=P)
            [:, q4 * 8:(q4 + 1) * 8, :])
        w2_chunks.append(w2t)

    w2_chunk_dma(0)
    w2_chunk_dma(1)

    # =====================================================================
    # P4 tail: out-proj + residual + LN2 + MLP on 256 tokens
    # =====================================================================
    tkb = tc.alloc_tile_pool(name="tail_keep", bufs=1)
    ofT = tkb.tile([128, DT, TAIL], BF16, name="ofT")
    for par in range(2):
        nc.sync.dma_start(
            ofT[:, :, par * 128:(par + 1) * 128],
            a2a_outs[par][:].rearrange("(jt p) t -> p jt t", p=P))
    h2 = tkb.tile([128, 2, D], BF16, name="h2")
    h2T = tkb.tile([128, DT, TAIL], BF16, name="h2T")
    zT = tkb.tile([128, MT, TAIL], BF16, name="zT")
    ys = tkb.tile([128, 2, D], F32, name="ys")

    with tc.tile_pool(name="op_ps", bufs=2, space="PSUM") as ops, \
         tc.tile_pool(name="tail_sb", bufs=2) as tsb:
        for tt2 in range(2):
            for nb in range(2):
                nsl = slice(nb * 512, (nb + 1) * 512)
                op_bank = ops.tile([128, 512], F32, name="op_bank")
                for jt in range(DT):
                    nc.tensor.matmul(op_bank[:],
                                     ofT[:, jt, tt2 * 128:(tt2 + 1) * 128],
                                     wo_sb[:, jt, nsl],
                                     start=(jt == 0), stop=(jt == DT - 1))
                nc.vector.tensor_tensor(x2_sb[:, tt2, nsl], op_bank[:],
                                        x2_sb[:, tt2, nsl], ALU.add)
        if (d := dump("x2", [128, 2 * D])) is not None:
            nc.sync.dma_start(d.ap().rearrange("p (n f) -> p n f", n=2),
                              x2_sb[:])

        # LN2 (token-major) -> h2 -> h2T via PE transposes
        ssum = tsb.tile([128, 2], F32, name="ssum")
        ssq = tsb.tile([128, 2], F32, name="ssq")
        sqd = tsb.tile([128, D], BF16, name="sqd")
        for tt2 in range(2):
            nc.vector.tensor_reduce(ssum[:, tt2:tt2 + 1], x2_sb[:, tt2, :],
                                    AX.X, ALU.add)
            nc.scalar.activation(sqd[:], x2_sb[:, tt2, :], AF.Square,
                                 accum_out=ssq[:, tt2:tt2 + 1])
        mu2 = tsb.tile([128, 2], F32, name="mu2")
        nc.vector.tensor_scalar_mul(mu2[:], ssum[:], 1.0 / D)
        var2 = tsb.tile([128, 2], F32, name="var2")
        nc.vector.tensor_tensor(var2[:], mu2[:], mu2[:], ALU.mult)
        ex22 = tsb.tile([128, 2], F32, name="ex22")
        nc.vector.tensor_scalar_mul(ex22[:], ssq[:], 1.0 / D)
        nc.vector.tensor_tensor(var2[:], ex22[:], var2[:], ALU.subtract)
        sd2 = tsb.tile([128, 2], F32, name="sd2")
        nc.scalar.activation(sd2[:], var2[:], AF.Sqrt, bias=eps_sb[:])
        r2 = tsb.tile([128, 2], F32, name="r2")
        nc.vector.reciprocal(r2[:], sd2[:])
        nmu2 = tsb.tile([128, 2], F32, name="nmu2")
        nc.vector.tensor_tensor(nmu2[:], r2[:], mu2[:], ALU.mult)
        nc.vector.tensor_scalar_mul(nmu2[:], nmu2[:], -1.0)
        for tt2 in range(2):
            nc.scalar.activation(h2[:, tt2, :], x2_sb[:, tt2, :], AF.Identity,
                                 bias=nmu2[:, tt2:tt2 + 1],
                                 scale=r2[:, tt2:tt2 + 1])

    with tc.tile_pool(name="h2t_ps", bufs=2, space="PSUM") as hps:
        for dt in range(DT):
            ph = hps.tile([128, 2, 128], BF16, name="ph")
            for tt2 in range(2):
                nc.tensor.transpose(ph[:, tt2, :],
                                    h2[:, tt2, dt * 128:(dt + 1) * 128],
                                    ident_sb)
            nc.scalar.activation(h2T[:, dt, :], ph[:], AF.Copy)

    # MLP1 m-major: y1T[mt] = sum_dt w1[dt,mt]^T h2T[dt]; gelu -> zT
    # (b1 + ln2_b@W1 is all-zero for this problem -- asserted on host)
    with tc.tile_pool(name="y1_ps", bufs=3, space="PSUM") as y1ps:
        for m2 in range(MT // 2):
            y1b = y1ps.tile([128, 2, TAIL], F32, name="y1b")
            for half in range(2):
                mt = m2 * 2 + half
                msl = slice(mt * 128, (mt + 1) * 128)
                for dt in range(DT):
                    nc.tensor.matmul(y1b[:, half, :], w1_sb[:, dt, msl],
                                     h2T[:, dt, :],
                                     start=(dt == 0), stop=(dt == DT - 1))
            nc.scalar.activation(zT[:, m2 * 2:(m2 + 1) * 2, :], y1b[:],
                                 AF.Gelu)

    w2_chunk_dma(2)
    w2_chunk_dma(3)

    # MLP2: y2[t,:] = sum_mt zT[mt]^T w2[mt]
    with tc.tile_pool(name="y2_ps", bufs=1, space="PSUM") as y2ps:
        y2_banks = [y2ps.tile([128, 512], F32, name=f"y2b{i}")
                    for i in range(4)]
        for mt in range(MT):
            w2t = w2_chunks[mt // 8]
            for tt2 in range(2):
                for nb in range(2):
                    nc.tensor.matmul(
                        y2_banks[tt2 * 2 + nb],
                        zT[:, mt, tt2 * 128:(tt2 + 1) * 128],
                        w2t[:, mt % 8, nb * 512:(nb + 1) * 512],
                        start=(mt == 0), stop=(mt == MT - 1))
        for tt2 in range(2):
            for nb in range(2):
                nsl = slice(nb * 512, (nb + 1) * 512)
                nc.vector.tensor_tensor(ys[:, tt2, nsl],
                                        y2_banks[tt2 * 2 + nb],
                                        x2_sb[:, tt2, nsl], ALU.add)
    nc.sync.dma_start(y_out.ap().rearrange("(n p) d -> p n d", p=P), ys[:])

    for pool in (tkb, w2p, w1p, persist, dram, const):
        pool.release()


def _build():
    nc = bacc.Bacc("TRN2", target_bir_lowering=False, debug=False,
                   num_devices=N_CORES)

    def din(name, shape, dt=BF16):
        return nc.dram_tensor(name, shape, dt, kind="ExternalInput")

    io = dict(
        xT=din("xT", [D, TOK]),
        x_res=din("x_res", [TAIL, D], F32),
        wqkvf=din("wqkvf", [D, 512]),
        wo=din("wo", [D, D]),
        w1=din("w1", [D, MLP]),
        w2=din("w2", [MLP, D]),
        bcols=din("bcols", [128, 4], F32),
        blob_f32=din("blob_f32", [128, 129], F32),
        blob_bf16=din("blob_bf16", [128, 3 * 128]),
        rowblob=din("rowblob", [1, 10 * 128]),
        y_out=nc.dram_tensor("y_out", [TAIL, D], F32, kind="ExternalOutput"),
    )

    dbg = [s for s in os.environ.get("GSA_DEBUG", "").split(",") if s]
    dbg_outs = {}

    def dump(name, shape, dt=F32):
        if name in dbg:
            t = nc.dram_tensor("dbg_" + name, shape, dt,
                               kind="ExternalOutput")
            dbg_outs[name] = t
            return t
        return None

    io["dump"] = dump
    with tile.TileContext(nc) as tcx:
        _emit(nc, tcx, io)
    nc.compile()
    return nc, sorted(dbg_outs)


def _host_prep(inputs):
    """Fold norms/biases into weights; build per-core in_maps."""
    f32 = np.float32
    bf16 = ml_dtypes.bfloat16
    x = np.asarray(inputs["hidden_states"], f32).reshape(TOK, D)
    ln1_w = np.asarray(inputs["ln1_w"], f32)
    ln1_b = np.asarray(inputs["ln1_b"], f32)
    ln2_w = np.asarray(inputs["ln2_w"], f32)
    ln2_b = np.asarray(inputs["ln2_b"], f32)
    gnorm = np.asarray(inputs["gnorm_w"], f32)
    Wq = np.asarray(inputs["Wq"], f32) * ln1_w[:, None]
    Wk = np.asarray(inputs["Wk"], f32) * ln1_w[:, None]
    Wv = np.asarray(inputs["Wv"], f32) * ln1_w[:, None]
    Wf = np.asarray(inputs["Wf"], f32) * ln1_w[:, None]
    bq = ln1_b @ np.asarray(inputs["Wq"], f32)
    bk = ln1_b @ np.asarray(inputs["Wk"], f32)
    bv = ln1_b @ np.asarray(inputs["Wv"], f32)
    bf_ = ln1_b @ np.asarray(inputs["Wf"], f32)
    assert np.allclose(bv, 0) and np.allclose(bf_, 0), \
        "v/f projection biases must be zero (ln1_b fold only done for q/k)"
    Wo = np.asarray(inputs["Wo"], f32) * np.tile(gnorm, H)[:, None]
    W1 = np.asarray(inputs["W1"], f32) * ln2_w[:, None]
    b1 = np.asarray(inputs["b1"], f32) + ln2_b @ np.asarray(inputs["W1"], f32)
    assert np.allclose(b1, 0), "MLP1 bias must be zero (not emitted on device)"
    W2 = np.asarray(inputs["W2"], f32)
    b2 = np.asarray(inputs["b2"], f32)

    tri = np.tril(np.ones((128, 128), f32))  # [t, tau] tau<=t
    xT = np.ascontiguousarray(x.T.astype(bf16))

    common = dict(
        xT=xT,
        wo=np.ascontiguousarray(Wo.astype(bf16)),
        w1=np.ascontiguousarray(W1.astype(bf16)),
        w2=np.ascontiguousarray(W2.astype(bf16)),
        blob_f32=np.ascontiguousarray(np.concatenate(
            [(-0.125 * tri).T, np.ones((128, 1), f32)], axis=1)),
        blob_bf16=np.ascontiguousarray(np.concatenate(
            [tri.T, np.eye(128, dtype=f32),
             np.kron(np.eye(2, dtype=f32), np.ones((64, 64), f32))],
            axis=1).astype(bf16)),
    )
    in_maps = []
    for r in range(N_CORES):
        jsl = slice(r * 128, (r + 1) * 128)  # 2 heads = 128 cols
        m = dict(common)
        m["x_res"] = np.ascontiguousarray(x[r * TAIL:(r + 1) * TAIL]
                                          + b2[None, :])
        wq, wk = Wq[:, jsl], Wk[:, jsl]
        wv, wf = Wv[:, jsl], Wf[:, jsl]
        m["wqkvf"] = np.ascontiguousarray(
            np.concatenate([wq, wk, wv, wf], axis=1).astype(bf16))
        rb = np.zeros((1, 10 * 128), f32)
        for i, w in enumerate((wq, wk, wv, wf)):
            rb[0, i * 128:(i + 1) * 128] = -w.sum(axis=0)
        rb[0, 8 * 128:9 * 128] = 1.0
        rb[0, 9 * 128] = 1.0
        m["rowblob"] = np.ascontiguousarray(rb.astype(bf16))
        bc = np.zeros((128, 4), f32)
        bc[:, 0], bc[:, 1] = bq[jsl], bk[jsl]
        m["bcols"] = np.ascontiguousarray(bc)
        in_maps.append(m)
    return in_maps


def kernel(**inputs):
    if "nc" not in _cache:
        _cache["nc"], _cache["dbg"] = _build()
    nc = _cache["nc"]
    in_maps = _host_prep(inputs)
    res = run_bass_kernel_spmd(nc, in_maps, core_ids=list(range(N_CORES)),
                               trace=bool(os.environ.get("GSA_TRACE")))
    _cache["last_results"] = res
    out = np.concatenate([res.results[r]["y_out"] for r in range(N_CORES)],
                         axis=0)
    return out.reshape(B, T, D)


# revision 44
# speedup vs baseline: 1.1416x; 1.0115x over previous
"""GSA video block kernel for 8 TRN2 NeuronCores (v2).

Sharding: head-parallel attention (2 heads/core) -> one AllToAll that
redistributes the RMS-normed head outputs from head-sharded to
token-sharded -> token-parallel tail (out-proj + LN2 + MLP with full
weights, 256 tokens/core).

v2 layout strategy: the host supplies x TRANSPOSED (xT [D, TOK]), so
LN1 never materializes h -- projections run feat-major from xT with a
rank-1 mean correction accumulated into the psum and a broadcast-rstd
multiply afterwards.  Token-major k/v/f come from tensor-engine
transposes (no DMA transposes anywhere).  The chunked scan processes
the 4 batches' chunk-c tiles as single [128,512]-wide ops, with all
state-independent prep emitted ahead of the serial state chain.  The
MLP computes y1 m-major so MLP2 needs no transpose.
"""

import os
import sys

import numpy as np
import ml_dtypes

if "/opt/trn_rl_repo" not in sys.path:
    sys.path.insert(0, "/opt/trn_rl_repo")

import concourse.bass as bass  # noqa: E402
import concourse.mybir as mybir  # noqa: E402
import concourse.tile as tile  # noqa: E402
from concourse import bacc  # noqa: E402
from concourse.bass_utils import run_bass_kernel_spmd  # noqa: E402

BF16 = mybir.dt.bfloat16
F32 = mybir.dt.float32
AF = mybir.ActivationFunctionType
ALU = mybir.AluOpType
AX = mybir.AxisListType

B, T, D = 4, 512, 1024
H, DK, DV, M = 16, 64, 64, 64
MLP = 4096
EPS = 1e-6

N_CORES = 8
C = 128                    # scan chunk length
NCH = T // C               # chunks per batch = 4
TOK = B * T                # 2048 flat tokens
TT = TOK // 128            # 16 token tiles
DT = D // 128              # 8 d tiles
MT = MLP // 128            # 32 mlp tiles
TAIL = TOK // N_CORES      # 256 tokens per core in the tail
RG = [list(range(N_CORES))]
P = 128

_cache = {}


def _emit(nc, tc, io):
    xT, x_res = io["xT"], io["x_res"]
    wqkvf, wo, w1, w2 = io["wqkvf"], io["wo"], io["w1"], io["w2"]
    blob_f32, blob_bf16, rowblob = io["blob_f32"], io["blob_bf16"], io["rowblob"]
    y_out, dump = io["y_out"], io["dump"]

    const = tc.alloc_tile_pool(name="const", bufs=1)
    dram = tc.alloc_tile_pool(name="dram", bufs=1, space="DRAM")

    # ---- warmup collective (prepay ncfw handshake) -----------------------
    wa_in = dram.tile([8, 128], BF16, name="wa_in")
    wa_out = dram.tile([8, 128], BF16, name="wa_out")
    nc.gpsimd.collective_compute("AllReduce", ALU.add, replica_groups=RG,
                                 ins=[wa_in.opt()], outs=[wa_out.opt()])
    wa2_in = dram.tile([8, 1024], BF16, name="wa2_in")
    wa2_out = dram.tile([8, 1024], BF16, name="wa2_out")
    nc.gpsimd.collective_compute("AllToAll", ALU.bypass, replica_groups=RG,
                                 ins=[wa2_in.opt()], outs=[wa2_out.opt()])

    # split A2A: half 1 ships chunk-parity-0 tokens (each dest core's first
    # 128 tail tokens), half 2 the parity-1 tokens
    a2a_ins = [dram.tile([128 * N_CORES, 128], BF16, name=f"a2a_in{i}")
               for i in range(2)]
    a2a_outs = [dram.tile([128 * N_CORES, 128], BF16, name=f"a2a_out{i}")
                for i in range(2)]

    # ---- constants into SBUF (SP queue) ---------------------------------
    cf = const.tile([128, 129], F32, name="cf")
    nc.sync.dma_start(cf[:], blob_f32.ap())
    ltriT_sb = cf[:, 0:128]
    onescol = cf[:, 128:129]                              # all ones
    cb = const.tile([128, 3, 128], BF16, name="cb")       # cmask|ident|bd128
    nc.sync.dma_start(cb[:], blob_bf16.ap().rearrange("p (n f) -> p n f", n=3))
    cmask_bc = cb[:, 0:1, :].to_broadcast([128, 4, 128])
    ident_sb, bd128_sb = cb[:, 1, :], cb[:, 2, :]
    rows = const.tile([1, 10, 128], BF16, name="rows")
    nc.sync.dma_start(rows[:], rowblob.ap().rearrange("o (n f) -> o n f", n=10))
    csq_r, csk_r, csv_r, csf_r = (rows[:, i, :] for i in range(4))
    ones_row = rows[:, 8, :]
    one1 = rows[:, 9, 0:1]
    bcol = const.tile([128, 4], F32, name="bcol")
    nc.sync.dma_start(bcol[:], io["bcols"].ap())
    eps_sb = const.tile([128, 1], F32)
    nc.vector.memset(eps_sb[:], EPS)
    eps2_sb = const.tile([128, 1], F32)
    nc.vector.memset(eps2_sb[:], 1e-30)

    # ---- persistent activation tensors (allocated below xp/wp in the
    # pool stack; released only at the very end) --------------------------
    persist = tc.alloc_tile_pool(name="persist", bufs=1)
    qT = persist.tile([128, TOK], BF16, name="qT")
    kT = persist.tile([128, TOK], BF16, name="kT")
    k_tm = persist.tile([128, TT, 128], BF16, name="k_tm")
    v_tm = persist.tile([128, TT, 128], BF16, name="v_tm")
    sp = persist.tile([128, TT, 128], F32, name="sp")
    s_tm = persist.tile([128, TT, 128], BF16, name="s_tm")
    onT = persist.tile([128, TOK], BF16, name="onT")

    # ---- bulk loads -----------------------------------------------------
    xp = tc.alloc_tile_pool(name="xp", bufs=1)
    xT_sb = xp.tile([128, DT, TOK], BF16, name="xT_sb")
    for dt in range(DT):
        nc.sync.dma_start(xT_sb[:, dt, :],
                          xT.ap().rearrange("(dt p) t -> p dt t", p=P)[:, dt, :])
    # w4 queued on Sync BEHIND the xT tiles so xT gets the DMA engines
    # first (stats start gates on it); wo/x_res load later (tail-only)
    wp = tc.alloc_tile_pool(name="wp", bufs=1)
    w4_sb = wp.tile([128, DT, 512], BF16, name="w4_sb")   # wq|wk|wv|wf cols
    nc.sync.dma_start(w4_sb[:], wqkvf.ap().rearrange("(dt p) j -> p dt j", p=P))
    wo_sb = const.tile([128, DT, D], BF16, name="wo_sb")
    x2_sb = const.tile([128, 2, D], F32, name="x2_sb")

    # =====================================================================
    # P1: LN1 stats from xT via matmul reduction (per 512-token group)
    # =====================================================================
    stats = tc.alloc_tile_pool(name="stats", bufs=1)
    mu_r = stats.tile([1, 4, 512], BF16, name="mu_r")
    sd_r = stats.tile([1, 4, 512], BF16, name="sd_r")
    rbc = stats.tile([128, 4, 512], F32, name="rbc")
    oc = stats.tile([128, 1], BF16, name="oc")
    nc.vector.memset(oc[:], 1.0 / D)
    with tc.tile_pool(name="sq_p", bufs=3) as sqp, \
         tc.tile_pool(name="st_ps", bufs=2, space="PSUM") as stps, \
         tc.tile_pool(name="rb_ps", bufs=2, space="PSUM") as rbps, \
         tc.tile_pool(name="st_sb", bufs=2) as stsb:
        for g in range(4):
            gsl = slice(g * 512, (g + 1) * 512)
            ps_mu = stps.tile([1, 512], F32, name="ps_mu")
            ps_e2 = stps.tile([1, 512], F32, name="ps_e2")
            for dt in range(DT):
                sq = sqp.tile([128, 512], BF16, name="sq")
                nc.vector.tensor_tensor(sq[:], xT_sb[:, dt, gsl],
                                        xT_sb[:, dt, gsl], ALU.mult)
                nc.tensor.matmul(ps_mu[:], oc[:], xT_sb[:, dt, gsl],
                                 start=(dt == 0), stop=(dt == DT - 1))
                nc.tensor.matmul(ps_e2[:], oc[:], sq[:],
                                 start=(dt == 0), stop=(dt == DT - 1))
            # var = e2 - mu^2 ; sd = sqrt(var+eps); rbc = 1/bcast(sd)
            nc.scalar.activation(mu_r[:, g, :], ps_mu[:], AF.Copy)
            var = stsb.tile([1, 512], F32, name="var")
            nc.vector.tensor_tensor(var[:], mu_r[:, g, :], mu_r[:, g, :],
                                    ALU.mult)
            nc.vector.tensor_tensor(var[:], ps_e2[:], var[:], ALU.subtract)
            nc.scalar.activation(sd_r[:, g, :], var[:], AF.Sqrt,
                                 bias=eps_sb[0:1, :])
            ps_rb = rbps.tile([128, 512], F32, name="ps_rb")
            nc.tensor.matmul(ps_rb[:], ones_row, sd_r[:, g, :],
                             start=True, stop=True)
            nc.vector.reciprocal_approx_fast(rbc[:, g, :], ps_rb[:])

    # =====================================================================
    # P2: projections, feat-major, with rank-1 mean correction
    # =====================================================================
    ffeat = tc.alloc_tile_pool(name="ffeat", bufs=1)
    f_ft = ffeat.tile([128, TOK], BF16, name="f_ft")
    with tc.tile_pool(name="pj_ps", bufs=3, space="PSUM") as pjps, \
         tc.tile_pool(name="pj_sb", bufs=3) as pjsb, \
         tc.tile_pool(name="tr_ps", bufs=2, space="PSUM") as trps:
        def proj_psum(jsl, cs_row, g):
            gsl = slice(g * 512, (g + 1) * 512)
            bank = pjps.tile([128, 512], F32, name="pjbank")
            for dt in range(DT):
                nc.tensor.matmul(bank[:], w4_sb[:, dt, jsl], xT_sb[:, dt, gsl],
                                 start=(dt == 0), stop=False)
            nc.tensor.matmul(bank[:], cs_row, mu_r[:, g, :],
                             start=False, stop=True)
            return bank

        for g in range(4):                      # q: silu -> qT
            bank = proj_psum(slice(0, 128), csq_r, g)
            gsl = slice(g * 512, (g + 1) * 512)
            yp = pjsb.tile([128, 512], BF16, name="yp")
            nc.vector.tensor_tensor(yp[:], bank[:], rbc[:, g, :], ALU.mult)
            nc.scalar.activation(qT[:, gsl], yp[:], AF.Silu,
                                 bias=bcol[:, 0:1])
        for g in range(4):                      # k: silu -> kT (+ k_tm below)
            bank = proj_psum(slice(128, 256), csk_r, g)
            gsl = slice(g * 512, (g + 1) * 512)
            yp = pjsb.tile([128, 512], BF16, name="yp")
            nc.vector.tensor_tensor(yp[:], bank[:], rbc[:, g, :], ALU.mult)
            nc.scalar.activation(kT[:, gsl], yp[:], AF.Silu,
                                 bias=bcol[:, 1:2])
        for g in range(4):
            pst = trps.tile([128, 4, 128], BF16, name="pst")
            for cc in range(4):
                ti = g * 4 + cc
                nc.tensor.transpose(pst[:, cc, :],
                                    kT[:, ti * 128:(ti + 1) * 128], ident_sb)
            nc.vector.tensor_copy(k_tm[:, g * 4:(g + 1) * 4, :], pst[:])
        for g in range(4):                      # v: mult only -> v_tm
            bank = proj_psum(slice(256, 384), csv_r, g)
            yp = pjsb.tile([128, 512], BF16, name="yp")
            nc.vector.tensor_tensor(yp[:], bank[:], rbc[:, g, :], ALU.mult)
            pst = trps.tile([128, 4, 128], BF16, name="pst")
            for cc in range(4):
                nc.tensor.transpose(pst[:, cc, :], yp[:, cc * 128:(cc + 1) * 128],
                                    ident_sb)
            nc.vector.tensor_copy(v_tm[:, g * 4:(g + 1) * 4, :], pst[:])
        for g in range(4):                      # f -> f_ft (feat-major)
            bank = proj_psum(slice(384, 512), csf_r, g)
            gsl = slice(g * 512, (g + 1) * 512)
            nc.vector.tensor_tensor(f_ft[:, gsl], bank[:], rbc[:, g, :],
                                    ALU.mult)
        # gates: sp = softplus(-f) = ln(1+exp(-f)); three passes so the
        # activation table switches exp -> ln -> exp only once each
        enf_a = pjsb.tile([128, TT, 128], F32, name="enf_a", bufs=1)
        for g in range(4):
            pst = trps.tile([128, 4, 128], BF16, name="pst")
            for cc in range(4):
                ti = g * 4 + cc
                nc.tensor.transpose(pst[:, cc, :],
                                    f_ft[:, ti * 128:(ti + 1) * 128], ident_sb)
            csl = slice(g * 4, (g + 1) * 4)
            nc.scalar.activation(enf_a[:, csl, :], pst[:], AF.Exp, scale=-1.0)
        for g in range(4):
            csl = slice(g * 4, (g + 1) * 4)
            nc.scalar.activation(sp[:, csl, :], enf_a[:, csl, :], AF.Ln,
                                 bias=1.0)
        for g in range(4):
            csl = slice(g * 4, (g + 1) * 4)
            e8 = pjsb.tile([128, 4, 128], BF16, name="e8")
            nc.scalar.activation(e8[:], sp[:, csl, :], AF.Exp, scale=-0.125)
            nc.vector.tensor_scalar(s_tm[:, csl, :], e8[:], -1.0, 1.0,
                                    ALU.mult, ALU.add)
    ffeat.release()
    stats.release()
    wp.release()
    xp.release()

    # tail loads (ACT queue, lands during the scan) + w1 prefetch on SYNC
    nc.scalar.dma_start(wo_sb[:], wo.ap().rearrange("(dt p) j -> p dt j", p=P))
    nc.scalar.dma_start(x2_sb[:], x_res.ap().rearrange("(n p) d -> p n d", p=P))
    w1p = tc.alloc_tile_pool(name="w1p", bufs=1)
    w1_sb = w1p.tile([128, DT, MLP], BF16, name="w1_sb")
    for q4 in range(4):
        msl = slice(q4 * 1024, (q4 + 1) * 1024)
        nc.sync.dma_start(
            w1_sb[:, :, msl],
            w1.ap().rearrange("(dt p) m -> p dt m", p=P)[:, :, msl])

    for nm, t_sb in (("qT", qT), ("kT", kT)):
        if (d := dump(nm, [128, TOK], BF16)) is not None:
            nc.sync.dma_start(d.ap(), t_sb[:])
    for nm, t_sb in (("k_tm", k_tm), ("v_tm", v_tm), ("s_tm", s_tm)):
        if (d := dump(nm, [128, TT * 128], BF16)) is not None:
            nc.sync.dma_start(d.ap().rearrange("p (n f) -> p n f", n=TT), t_sb[:])
    if (d := dump("sp", [128, TT * 128])) is not None:
        nc.sync.dma_start(d.ap().rearrange("p (n f) -> p n f", n=TT), sp[:])

    # =====================================================================
    # P3: chunked scan.  Token tile index = b*4+c.  Group = fixed c, 4 b's.
    # =====================================================================
    scank = tc.alloc_tile_pool(name="scank", bufs=1)
    # ping-pong state buffers: slot c%2 holds the state AFTER chunk c, so
    # state updates can be emitted ahead of the chunk's softmax chain
    Kst = scank.tile([128, 2, 4, 64], BF16, name="Kst")  # [(2h dk), pp, b, m]
    Vst = scank.tile([128, 2, 4, 64], BF16, name="Vst")  # [(2h m), pp, b, dv]
    lam_a = scank.tile([128, NCH, 4, 128], BF16, name="lam_a")
    lamT_a = scank.tile([128, NCH, 4, 128], BF16, name="lamT_a")
    stil_a = scank.tile([128, NCH, 4, 128], BF16, name="stil_a")
    stT_a = scank.tile([128, NCH, 4, 128], BF16, name="stT_a")
    am_a = scank.tile([128, NCH, 2, 4, 128], BF16, name="am_a")
    dk_a = scank.tile([128, NCH, 4, 64], BF16, name="dk_a")   # [(2h dk), b, m]
    dv_a = scank.tile([128, NCH, 4, 64], BF16, name="dv_a")   # [(2h m), b, dv]
    lbc_a = scank.tile([128, NCH, 4, 128], BF16, name="lbc_a")
    dec_a = scank.tile([128, NCH, 4], BF16, name="dec_a")

    def cgv(t_sb, c):
        # [128, TT, f] -> [128, 4b, f] strided view for chunk c
        return t_sb[:].rearrange("p (b c) f -> p c b f", c=NCH)[:, c]

    # serial-phase + rms psum pools FIRST so their banks are disjoint from
    # the prep rotation's
    with tc.tile_pool(name="se_ps", bufs=1, space="PSUM") as seps, \
         tc.tile_pool(name="rms_ps", bufs=1, space="PSUM") as rps, \
         tc.tile_pool(name="pr_ps", bufs=2, space="PSUM") as prps, \
         tc.tile_pool(name="pr_sb", bufs=2) as prsb, \
         tc.tile_pool(name="rms_sb", bufs=2) as rsb, \
         tc.tile_pool(name="se_sb", bufs=2) as sesb:

        def rms_and_ship(par):
            # normalize + pack + AllToAll for tiles of chunk parity `par`
            onh = onT[:].rearrange("p (x e f) -> p e x f", x=8, e=2)[:, par]
            for q2 in range(2):
                qv = onh[:, q2 * 4:(q2 + 1) * 4, :]
                sqo = rsb.tile([128, 4, 128], BF16, name="sqo")
                nc.vector.tensor_tensor(sqo[:], qv, qv, ALU.mult)
                ps_ss = rps.tile([128, 512], F32, name="ps_ss")
                nc.tensor.matmul(ps_ss[:], bd128_sb, sqo[:],
                                 start=True, stop=True)
                srt = rsb.tile([128, 512], F32, name="srt")
                nc.scalar.activation(srt[:], ps_ss[:], AF.Sqrt,
                                     bias=eps2_sb[:], scale=1.0 / DV)
                rro = rsb.tile([128, 4, 128], F32, name="rro")
                nc.vector.reciprocal_approx_fast(
                    rro[:].rearrange("p b f -> p (b f)"), srt[:])
                nc.vector.tensor_tensor(qv, qv, rro[:], ALU.mult)
            nc.sync.dma_start(
                a2a_ins[par][:].rearrange("(r p) t -> p r t", p=P), onh)
            nc.gpsimd.collective_compute(
                "AllToAll", ALU.bypass, replica_groups=RG,
                ins=[a2a_ins[par].opt()], outs=[a2a_outs[par].opt()])

        # ---------------- prep (state-independent), all c ----------------
        for c in range(NCH):
            ps_cs = prps.tile([128, 4, 128], F32, name="ps_cs", tag="pband")
            nc.tensor.matmul(ps_cs[:], ltriT_sb[:], cgv(sp, c),
                             start=True, stop=True)
            nc.scalar.activation(lam_a[:, c], ps_cs[:], AF.Exp)
            en4 = prsb.tile([128, 4, 128], BF16, name="en4")
            nc.scalar.activation(en4[:], ps_cs[:], AF.Exp, scale=-1.0)
            ps_ct = prps.tile([1, 4, 128], F32, name="ps_ct", tag="pband")
            nc.tensor.matmul(ps_ct[:], onescol, cgv(sp, c),
                             start=True, stop=True)
            lamCr = prsb.tile([1, 4, 128], BF16, name="lamCr")
            nc.scalar.activation(lamCr[:], ps_ct[:], AF.Exp, scale=-0.125)
            ps_lb = prps.tile([128, 4, 128], F32, name="ps_lb", tag="pband")
            nc.tensor.matmul(ps_lb[:], ones_row,
                             lamCr[:].rearrange("o b f -> o (b f)"),
                             start=True, stop=True)
            nc.scalar.activation(lbc_a[:, c], ps_lb[:], AF.Copy)
            nc.vector.tensor_tensor(stil_a[:, c], cgv(s_tm, c), en4[:],
                                    ALU.mult)
            s2 = prsb.tile([128, 4, 128], BF16, name="s2")
            nc.vector.tensor_tensor(s2[:], stil_a[:, c], ps_lb[:], ALU.mult)
            ps_dc = prps.tile([128, 4], F32, name="ps_dc", tag="pband")
            for b in range(4):
                nc.tensor.matmul(ps_dc[:, b:b + 1], lamCr[:, b, :], one1,
                                 start=True, stop=True)
            nc.scalar.activation(dec_a[:, c], ps_dc[:], AF.Copy)
            for h in range(2):
                hs = slice(h * 64, (h + 1) * 64)
                ps_a = prps.tile([128, 4, 128], F32, name="ps_a", tag="pband")
                for b in range(4):
                    tsl = slice((b * 4 + c) * 128, (b * 4 + c + 1) * 128)
                    nc.tensor.matmul(ps_a[:, b, :], kT[hs, tsl], qT[hs, tsl],
                                     start=True, stop=True)
                nc.vector.tensor_tensor(am_a[:, c, h], ps_a[:], cmask_bc,
                                        ALU.mult)
            ps_st = prps.tile([128, 4, 128], BF16, name="ps_st", tag="pband")
            for b in range(4):
                nc.tensor.transpose(ps_st[:, b, :], stil_a[:, c, b, :],
                                    ident_sb)
            nc.scalar.activation(stT_a[:, c], ps_st[:], AF.Copy)
            ps_lt = prps.tile([128, 4, 128], BF16, name="ps_lt", tag="pband")
            for b in range(4):
                nc.tensor.transpose(ps_lt[:, b, :], lam_a[:, c, b, :],
                                    ident_sb)
            nc.scalar.activation(lamT_a[:, c], ps_lt[:], AF.Copy)
            ps_dk = prps.tile([128, 4, 64], F32, name="ps_dk", tag="pband")
            for h in range(2):
                hs = slice(h * 64, (h + 1) * 64)
                for b in range(4):
                    bi = b * 4 + c
                    nc.tensor.matmul(ps_dk[hs, b, :], k_tm[:, bi, hs],
                                     s2[:, b, hs], start=True, stop=True)
            nc.scalar.activation(dk_a[:, c], ps_dk[:], AF.Copy)
            ps_dv = prps.tile([128, 4, 64], F32, name="ps_dv", tag="pband")
            for h in range(2):
                hs = slice(h * 64, (h + 1) * 64)
                for b in range(4):
                    bi = b * 4 + c
                    nc.tensor.matmul(ps_dv[hs, b, :], s2[:, b, hs],
                                     v_tm[:, bi, hs], start=True, stop=True)
            nc.scalar.activation(dv_a[:, c], ps_dv[:], AF.Copy)

        # ---------------- serial state chain -----------------------------
        # state updates are emitted FIRST per chunk (into the ping-pong
        # slot) so the V-queue advances the state chain ahead of each
        # chunk's softmax chain; the four chunks' output chains pipeline.
        for c in range(NCH):
            first = (c == 0)
            nw, od = c % 2, (c - 1) % 2
            if first:
                nc.vector.tensor_copy(Kst[:, 0], dk_a[:, c])
                nc.vector.tensor_copy(Vst[:, 0], dv_a[:, c])
            else:
                for h in range(2):
                    hs = slice(h * 64, (h + 1) * 64)
                    nc.vector.tensor_tensor(Kst[hs, nw], Kst[hs, od],
                                            lbc_a[hs, c, :, hs], ALU.mult)
                nc.vector.tensor_tensor(Kst[:, nw], Kst[:, nw], dk_a[:, c],
                                        ALU.add)
                nc.vector.tensor_tensor(
                    Vst[:, nw], Vst[:, od],
                    dec_a[:, c].rearrange("p (b o) -> p b o", b=4)
                    .to_broadcast([128, 4, 64]),
                    ALU.mult)
                nc.vector.tensor_tensor(Vst[:, nw], Vst[:, nw], dv_a[:, c],
                                        ALU.add)
            ps_ok = seps.tile([128, 4, 2, 64], F32, name="ps_ok", bufs=2)
            for h in range(2):
                hs = slice(h * 64, (h + 1) * 64)
                for b in range(4):
                    tsl = slice((b * 4 + c) * 128, (b * 4 + c + 1) * 128)
                    if not first:
                        nc.tensor.matmul(ps_ok[:, b, h, :], qT[hs, tsl],
                                         Kst[hs, od, b, :],
                                         start=True, stop=False)
                    nc.tensor.matmul(ps_ok[:, b, h, :], am_a[:, c, h, b, :],
                                     stil_a[:, c, b, hs],
                                     start=first, stop=True)
            # slot-softmax WITHOUT the denominator: the per-(t,h) positive
            # scale cancels in the downstream RMS norm over dv (eps there
            # is set tiny so the cancellation is exact to fp precision)
            oksc = sesb.tile([128, 4, 128], F32, name="oksc")
            nc.vector.tensor_tensor(
                oksc[:], ps_ok[:].rearrange("p b h f -> p b (h f)"),
                lam_a[:, c], ALU.mult)
            ex = sesb.tile([128, 4, 128], BF16, name="ex")
            nc.scalar.activation(ex[:], oksc[:], AF.Exp, scale=0.125)
            ps_pt = seps.tile([128, 4, 128], BF16, name="ps_pt")
            for b in range(4):
                nc.tensor.transpose(ps_pt[:, b, :], ex[:, b, :], ident_sb)
            plT = sesb.tile([128, 4, 128], BF16, name="plT")
            nc.vector.tensor_tensor(plT[:], ps_pt[:], lamT_a[:, c], ALU.mult)
            ps_o = seps.tile([128, 4, 128], F32, name="ps_o")
            for h in range(2):
                hs = slice(h * 64, (h + 1) * 64)
                ps_b2 = seps.tile([128, 4, 128], F32, name="ps_b2")
                for b in range(4):
                    nc.tensor.matmul(ps_b2[:, b, :], stT_a[hs, c, b, :],
                                     plT[hs, b, :], start=True, stop=True)
                b2m = sesb.tile([128, 4, 128], BF16, name="b2m")
                nc.vector.tensor_tensor(b2m[:], ps_b2[:], cmask_bc, ALU.mult)
                for b in range(4):
                    bi = b * 4 + c
                    if not first:
                        nc.tensor.matmul(ps_o[hs, b, :], Vst[hs, od, b, :],
                                         plT[hs, b, :], start=True, stop=False)
                    nc.tensor.matmul(ps_o[hs, b, :], v_tm[:, bi, hs],
                                     b2m[:, b, :], start=first, stop=True)
            onv = onT[:].rearrange("p (b c f) -> p c b f", b=4, c=NCH)
            nc.scalar.activation(onv[:, c], ps_o[:], AF.Copy)
            if c == 2:
                rms_and_ship(0)
        rms_and_ship(1)

    if (d := dump("onT", [128, TOK], BF16)) is not None:
        nc.sync.dma_start(d.ap(), onT[:])
    scank.release()

    # w2 prefetch (ACT queue): double-buffered chunks; first two issued
    # here, last two after MLP1 emission (their WAR deps are MLP2 reads,
    # which must already be behind them in the Act queue to avoid a
    # head-of-line deadlock)
    w2p = tc.alloc_tile_pool(name="w2p", bufs=2)
    w2_chunks = []

    def w2_chunk_dma(q4):
        w2t = w2p.tile([128, 8, D], BF16, name="w2t")
        nc.scalar.dma_start(
            w2t[:], w2.ap().rearrange("(n p) d -> p n d", p=P)
            [:, q4 * 8:(q4 + 1) * 8, :])
        w2_chunks.append(w2t)

    w2_chunk_dma(0)
    w2_chunk_dma(1)

    # =====================================================================
    # P4 tail: out-proj + residual + LN2 + MLP on 256 tokens
    # =====================================================================
    tkb = tc.alloc_tile_pool(name="tail_keep", bufs=1)
    ofT = tkb.tile([128, DT, TAIL], BF16, name="ofT")
    for par in range(2):
        nc.sync.dma_start(
            ofT[:, :, par * 128:(par + 1) * 128],
            a2a_outs[par][:].rearrange("(jt p) t -> p jt t", p=P))
    h2 = tkb.tile([128, 2, D], BF16, name="h2")
    h2T = tkb.tile([128, DT, TAIL], BF16, name="h2T")
    zT = tkb.tile([128, MT, TAIL], BF16, name="zT")
    ys = tkb.tile([128, 2, D], F32, name="ys")

    with tc.tile_pool(name="op_ps", bufs=2, space="PSUM") as ops, \
         tc.tile_pool(name="tail_sb", bufs=2) as tsb:
        for tt2 in range(2):
            for nb in range(2):
                nsl = slice(nb * 512, (nb + 1) * 512)
                op_bank = ops.tile([128, 512], F32, name="op_bank")
                for jt in range(DT):
                    nc.tensor.matmul(op_bank[:],
                                     ofT[:, jt, tt2 * 128:(tt2 + 1) * 128],
                                     wo_sb[:, jt, nsl],
                                     start=(jt == 0), stop=(jt == DT - 1))
                nc.vector.tensor_tensor(x2_sb[:, tt2, nsl], op_bank[:],
                                        x2_sb[:, tt2, nsl], ALU.add)
        if (d := dump("x2", [128, 2 * D])) is not None:
            nc.sync.dma_start(d.ap().rearrange("p (n f) -> p n f", n=2),
                              x2_sb[:])

        # LN2 (token-major) -> h2 -> h2T via PE transposes
        ssum = tsb.tile([128, 2], F32, name="ssum")
        ssq = tsb.tile([128, 2], F32, name="ssq")
        sqd = tsb.tile([128, D], BF16, name="sqd")
        for tt2 in range(2):
            nc.vector.tensor_reduce(ssum[:, tt2:tt2 + 1], x2_sb[:, tt2, :],
                                    AX.X, ALU.add)
            nc.scalar.activation(sqd[:], x2_sb[:, tt2, :], AF.Square,
                                 accum_out=ssq[:, tt2:tt2 + 1])
        mu2 = tsb.tile([128, 2], F32, name="mu2")
        nc.vector.tensor_scalar_mul(mu2[:], ssum[:], 1.0 / D)
        var2 = tsb.tile([128, 2], F32, name="var2")
        nc.vector.tensor_tensor(var2[:], mu2[:], mu2[:], ALU.mult)
        ex22 = tsb.tile([128, 2], F32, name="ex22")
        nc.vector.tensor_scalar_mul(ex22[:], ssq[:], 1.0 / D)
        nc.vector.tensor_tensor(var2[:], ex22[:], var2[:], ALU.subtract)
        sd2 = tsb.tile([128, 2], F32, name="sd2")
        nc.scalar.activation(sd2[:], var2[:], AF.Sqrt, bias=eps_sb[:])
        r2 = tsb.tile([128, 2], F32, name="r2")
        nc.vector.reciprocal(r2[:], sd2[:])
        nmu2 = tsb.tile([128, 2], F32, name="nmu2")
        nc.vector.tensor_tensor(nmu2[:], r2[:], mu2[:], ALU.mult)
        nc.vector.tensor_scalar_mul(nmu2[:], nmu2[:], -1.0)
        for tt2 in range(2):
            nc.scalar.activation(h2[:, tt2, :], x2_sb[:, tt2, :], AF.Identity,
                                 bias=nmu2[:, tt2:tt2 + 1],
                                 scale=r2[:, tt2:tt2 + 1])

    with tc.tile_pool(name="h2t_ps", bufs=2, space="PSUM") as hps:
        for dt in range(DT):
            ph = hps.tile([128, 2, 128], BF16, name="ph")
            for tt2 in range(2):
                nc.tensor.transpose(ph[:, tt2, :],
                                    h2[:, tt2, dt * 128:(dt + 1) * 128],
                                    ident_sb)
            nc.scalar.activation(h2T[:, dt, :], ph[:], AF.Copy)

    # MLP1 m-major: y1T[mt] = sum_dt w1[dt,mt]^T h2T[dt]; gelu -> zT
    # (b1 + ln2_b@W1 is all-zero for this problem -- asserted on host)
    with tc.tile_pool(name="y1_ps", bufs=3, space="PSUM") as y1ps:
        for m2 in range(MT // 2):
            y1b = y1ps.tile([128, 2, TAIL], F32, name="y1b")
            for half in range(2):
                mt = m2 * 2 + half
                msl = slice(mt * 128, (mt + 1) * 128)
                for dt in range(DT):
                    nc.tensor.matmul(y1b[:, half, :], w1_sb[:, dt, msl],
                                     h2T[:, dt, :],
                                     start=(dt == 0), stop=(dt == DT - 1))
            nc.scalar.activation(zT[:, m2 * 2:(m2 + 1) * 2, :], y1b[:],
                                 AF.Gelu)

    w2_chunk_dma(2)
    w2_chunk_dma(3)

    # MLP2: y2[t,:] = sum_mt zT[mt]^T w2[mt]
    with tc.tile_pool(name="y2_ps", bufs=1, space="PSUM") as y2ps:
        y2_banks = [y2ps.tile([128, 512], F32, name=f"y2b{i}")
                    for i in range(4)]
        for mt in range(MT):
            w2t = w2_chunks[mt // 8]
            for tt2 in range(2):
                for nb in range(2):
                    nc.tensor.matmul(
                        y2_banks[tt2 * 2 + nb],
                        zT[:, mt, tt2 * 128:(tt2 + 1) * 128],
                        w2t[:, mt % 8, nb * 512:(nb + 1) * 512],
                        start=(mt == 0), stop=(mt == MT - 1))
        for tt2 in range(2):
            for nb in range(2):
                nsl = slice(nb * 512, (nb + 1) * 512)
                nc.vector.tensor_tensor(ys[:, tt2, nsl],
                                        y2_banks[tt2 * 2 + nb],
                                        x2_sb[:, tt2, nsl], ALU.add)
    nc.sync.dma_start(y_out.ap().rearrange("(n p) d -> p n d", p=P), ys[:])

    for pool in (tkb, w2p, w1p, persist, dram, const):
        pool.release()


def _build():
    nc = bacc.Bacc("TRN2", target_bir_lowering=False, debug=False,
                   num_devices=N_CORES)

    def din(name, shape, dt=BF16):
        return nc.dram_tensor(name, shape, dt, kind="ExternalInput")

    io = dict(
        xT=din("xT", [D, TOK]),
        x_res=din("x_res", [TAIL, D], F32),
        wqkvf=din("wqkvf", [D, 512]),
        wo=din("wo", [D, D]),
        w1=din("w1", [D, MLP]),
        w2=din("w2", [MLP, D]),
        bcols=din("bcols", [128, 4], F32),
        blob_f32=din("blob_f32", [128, 129], F32),
        blob_bf16=din("blob_bf16", [128, 3 * 128]),
        rowblob=din("rowblob", [1, 10 * 128]),
        y_out=nc.dram_tensor("y_out", [TAIL, D], F32, kind="ExternalOutput"),
    )

    dbg = [s for s in os.environ.get("GSA_DEBUG", "").split(",") if s]
    dbg_outs = {}

    def dump(name, shape, dt=F32):
        if name in dbg:
            t = nc.dram_tensor("dbg_" + name, shape, dt,
                               kind="ExternalOutput")
            dbg_outs[name] = t
            return t
        return None

    io["dump"] = dump
    with tile.TileContext(nc) as tcx:
        _emit(nc, tcx, io)
    nc.compile()
    return nc, sorted(dbg_outs)


def _host_prep(inputs):
    """Fold norms/biases into weights; build per-core in_maps."""
    f32 = np.float32
    bf16 = ml_dtypes.bfloat16
    x = np.asarray(inputs["hidden_states"], f32).reshape(TOK, D)
    ln1_w = np.asarray(inputs["ln1_w"], f32)
    ln1_b = np.asarray(inputs["ln1_b"], f32)
    ln2_w = np.asarray(inputs["ln2_w"], f32)
    ln2_b = np.asarray(inputs["ln2_b"], f32)
    gnorm = np.asarray(inputs["gnorm_w"], f32)
    Wq = np.asarray(inputs["Wq"], f32) * ln1_w[:, None]
    Wk = np.asarray(inputs["Wk"], f32) * ln1_w[:, None]
    Wv = np.asarray(inputs["Wv"], f32) * ln1_w[:, None]
    Wf = np.asarray(inputs["Wf"], f32) * ln1_w[:, None]
    bq = ln1_b @ np.asarray(inputs["Wq"], f32)
    bk = ln1_b @ np.asarray(inputs["Wk"], f32)
    bv = ln1_b @ np.asarray(inputs["Wv"], f32)
    bf_ = ln1_b @ np.asarray(inputs["Wf"], f32)
    assert np.allclose(bv, 0) and np.allclose(bf_, 0), \
        "v/f projection biases must be zero (ln1_b fold only done for q/k)"
    Wo = np.asarray(inputs["Wo"], f32) * np.tile(gnorm, H)[:, None]
    W1 = np.asarray(inputs["W1"], f32) * ln2_w[:, None]
    b1 = np.asarray(inputs["b1"], f32) + ln2_b @ np.asarray(inputs["W1"], f32)
    assert np.allclose(b1, 0), "MLP1 bias must be zero (not emitted on device)"
    W2 = np.asarray(inputs["W2"], f32)
    b2 = np.asarray(inputs["b2"], f32)

    tri = np.tril(np.ones((128, 128), f32))  # [t, tau] tau<=t
    xT = np.ascontiguousarray(x.T.astype(bf16))

    common = dict(
        xT=xT,
        wo=np.ascontiguousarray(Wo.astype(bf16)),
        w1=np.ascontiguousarray(W1.astype(bf16)),
        w2=np.ascontiguousarray(W2.astype(bf16)),
        blob_f32=np.ascontiguousarray(np.concatenate(
            [(-0.125 * tri).T, np.ones((128, 1), f32)], axis=1)),
        blob_bf16=np.ascontiguousarray(np.concatenate(
            [tri.T, np.eye(128, dtype=f32),
             np.kron(np.eye(2, dtype=f32), np.ones((64, 64), f32))],
            axis=1).astype(bf16)),
    )
    in_maps = []
    for r in range(N_CORES):
        jsl = slice(r * 128, (r + 1) * 128)  # 2 heads = 128 cols
        m = dict(common)
        m["x_res"] = np.ascontiguousarray(x[r * TAIL:(r + 1) * TAIL]
                                          + b2[None, :])
        wq, wk = Wq[:, jsl], Wk[:, jsl]
        wv, wf = Wv[:, jsl], Wf[:, jsl]
        m["wqkvf"] = np.ascontiguousarray(
            np.concatenate([wq, wk, wv, wf], axis=1).astype(bf16))
        rb = np.zeros((1, 10 * 128), f32)
        for i, w in enumerate((wq, wk, wv, wf)):
            rb[0, i * 128:(i + 1) * 128] = -w.sum(axis=0)
        rb[0, 8 * 128:9 * 128] = 1.0
        rb[0, 9 * 128] = 1.0
        m["rowblob"] = np.ascontiguousarray(rb.astype(bf16))
        bc = np.zeros((128, 4), f32)
        bc[:, 0], bc[:, 1] = bq[jsl], bk[jsl]
        m["bcols"] = np.ascontiguousarray(bc)
        in_maps.append(m)
    return in_maps


def kernel(**inputs):
    if "nc" not in _cache:
        _cache["nc"], _cache["dbg"] = _build()
    nc = _cache["nc"]
    in_maps = _host_prep(inputs)
    res = run_bass_kernel_spmd(nc, in_maps, core_ids=list(range(N_CORES)),
                               trace=bool(os.environ.get("GSA_TRACE")))
    _cache["last_results"] = res
    out = np.concatenate([res.results[r]["y_out"] for r in range(N_CORES)],
                         axis=0)
    return out.reshape(B, T, D)
